# revision 1
# baseline (speedup 1.0000x reference)
"""2-layer GAT + mean-pool + log_softmax on 8 TRN2 NeuronCores — single launch.

Design (dst-sharded, band-ELL layout, super-group batched):
  - T1 = [s_src(4)|s_dst(4)|h1(64)] bf16 rows built sharded (12544/core,
    matmul with per-chunk stationary xT so rows come out node-major, no
    transposes), AllGathered to a full node-indexed table.
  - Each core owns 12500 dst nodes, degree-sorted into 98 bands of 128
    (one dst per partition, edges along free dim). Bands are batched into
    equal-width super-groups (SG x L <= 96 slots) so each op covers several
    bands in ONE instruction. Per group: one indirect DMA gathers
    [128, SG*L, 72] source rows; softmax is exp(lrelu(a+b)) = max(exp(y),
    exp(0.2y)); aggregation is a trailing-axis tensor_reduce.
  - Layer-1 output rows accumulate into an SBUF-resident T2 slot tile; ONE
    bulk indirect scatter (by node id) + AllGather feeds the same band
    pipeline for layer 2 and a 64-graph one-hot pooling matmul in PSUM.
  - Host: sum 8 partial pools, mean, +b2, log_softmax.
Pad slots gather table row `n` which holds s_src=-100 => exp ~ e^-20 ~ 0.
"""
import contextlib
import os
import numpy as np
import ml_dtypes

import jax
from jax.sharding import Mesh, PartitionSpec, NamedSharding
from jax.experimental.shard_map import shard_map

import concourse.bass as bass
import concourse.bacc as bacc
import concourse.mybir as mybir
import concourse.tile as tile
from concourse.bass2jax import _bass_exec_p, install_neuronx_cc_hook, partition_id_tensor

DT = mybir.dt
AF = mybir.ActivationFunctionType
OP = mybir.AluOpType
AX = mybir.AxisListType
BF16 = ml_dtypes.bfloat16
P = 128
NEG = 0.2
SLOT_BUDGET = int(os.environ.get("KV_SLOTS", "96"))
SG_MAX = int(os.environ.get("KV_SGMAX", "8"))

FULL = dict(n=100000, ncores=8, npc=12500, nband=98, shard=12544, ngraph=64)


class _PhaseStop(Exception):
    pass


# ---------------------------------------------------------------- host prep
def prep_edges(edge_index, cfg):
    n, ncores, npc, nband = cfg["n"], cfg["ncores"], cfg["npc"], cfg["nband"]
    pad_row = n
    src = np.asarray(edge_index[0], dtype=np.int64)
    dst = np.asarray(edge_index[1], dtype=np.int64)
    loop = np.arange(n, dtype=np.int64)
    src = np.concatenate([src, loop])
    dst = np.concatenate([dst, loop])

    core = dst // npc
    ldst = dst - core * npc

    deg = np.zeros((ncores, npc), dtype=np.int64)
    np.add.at(deg, (core, ldst), 1)

    nslot = nband * P
    perm = np.full((ncores, nslot), -1, dtype=np.int64)
    slot_of = np.zeros((ncores, npc), dtype=np.int64)
    for c in range(ncores):
        order = np.argsort(-deg[c], kind="stable")
        perm[c, :npc] = order
        slot_of[c, order] = np.arange(npc)

    degs_sorted = np.zeros((ncores, nslot), dtype=np.int64)
    for c in range(ncores):
        degs_sorted[c, :npc] = deg[c, perm[c, :npc]]
    band_max = degs_sorted.reshape(ncores, nband, P).max(axis=2)
    L = np.maximum(band_max.max(axis=0), 1).astype(np.int64)

    # adaptive super-groups: consecutive bands, equal width L[t0],
    # SG*L <= SLOT_BUDGET (L is non-increasing so L[t0] is the max)
    Ltrue = [int(x) for x in L]   # pre-equalization widths (gather bound)
    groups = []  # (t0, sg, lg)
    t0 = 0
    while t0 < nband:
        lg = int(L[t0])
        sg = 1
        while (t0 + sg < nband and sg < SG_MAX
               and (sg + 1) * lg <= SLOT_BUDGET):
            sg += 1
        groups.append((t0, sg, lg))
        L[t0:t0 + sg] = lg
        t0 += sg

    off = np.concatenate([[0], np.cumsum(L)[:-1]])
    SL = int(L.sum())

    # slot-space row id of every node: core*nslot + slot_of(node)
    # (tables are stored in degree-sorted slot order; pad -> slot 12543 of
    # core 0, a guaranteed dummy slot patched to s=-100)
    src_core = src // npc
    src_slot = (src_core * nslot + slot_of[src_core, src - src_core * npc])
    pad_slot = nslot - 1

    srcidx = np.full((ncores, P, SL), pad_slot, dtype=np.int32)
    slot = slot_of[core, ldst]
    band = slot // P
    part = slot % P
    key = core * nslot + slot
    ordk = np.argsort(key, kind="stable")
    key_s = key[ordk]
    starts = np.flatnonzero(np.r_[True, key_s[1:] != key_s[:-1]])
    reps = np.diff(np.r_[starts, len(key_s)])
    run = np.arange(len(key_s)) - np.repeat(starts, reps)
    col = np.empty(len(key_s), dtype=np.int64)
    col[ordk] = off[band[ordk]] + run
    srcidx[core, part, col] = src_slot.astype(np.int32)

    # node id per slot (for xts permutation and batchloc); -1 for dummies
    slot_node = np.full((ncores, nslot), -1, dtype=np.int64)
    for c in range(ncores):
        valid = perm[c] >= 0
        slot_node[c] = np.where(valid, perm[c] + c * npc, -1)

    return dict(srcidx=srcidx, slot_node=slot_node,
                L=[int(x) for x in L], off=[int(x) for x in off], SL=SL,
                groups=groups, Ltrue=Ltrue)


def build_weights(W1, a1s, a1d, W2, a2s, a2d):
    W1T = np.asarray(W1, np.float32).T          # [F_in, 64]
    fin = W1T.shape[0]
    wf = np.zeros((fin, 80), np.float32)
    for h in range(4):
        wf[:, h] = W1T[:, 16 * h:16 * (h + 1)] @ np.asarray(a1s, np.float32)[h]
        wf[:, 4 + h] = W1T[:, 16 * h:16 * (h + 1)] @ np.asarray(a1d, np.float32)[h]
    wf[:, 8:72] = W1T
    W2T = np.asarray(W2, np.float32).T          # [64, 10]
    w2c = np.zeros((64, 16), np.float32)
    w2c[:, 0] = W2T @ np.asarray(a2s, np.float32).reshape(-1)
    w2c[:, 1] = W2T @ np.asarray(a2d, np.float32).reshape(-1)
    w2c[:, 2:12] = W2T
    return wf, w2c


# ---------------------------------------------------------------- module
def build_module(cfg, ep, use_cc=True, has_b1=True, phases="ABCDE"):
    n, ncores, npc = cfg["n"], cfg["ncores"], cfg["npc"]
    nband, shard, ng = cfg["nband"], cfg["shard"], cfg["ngraph"]
    L, off, SL, groups = ep["L"], ep["off"], ep["SL"], ep["groups"]
    Ltrue = ep.get("Ltrue", L)
    nt = shard * ncores
    pad_row = n
    nc = bacc.Bacc("TRN2", target_bir_lowering=False,
                   num_devices=ncores if use_cc else 1)

    xts = nc.dram_tensor("xts", [P, shard], DT.bfloat16, kind="ExternalInput")
    wf = nc.dram_tensor("wf", [P, 80], DT.bfloat16, kind="ExternalInput")
    w2c = nc.dram_tensor("w2c", [64, 16], DT.bfloat16, kind="ExternalInput")
    srci = nc.dram_tensor("srci", [P, SL], DT.int32, kind="ExternalInput")
    bli = nc.dram_tensor("bli", [P, nband], DT.bfloat16, kind="ExternalInput")
    iog = nc.dram_tensor("iog", [P, ng], DT.bfloat16, kind="ExternalInput")
    b1i = nc.dram_tensor("b1i", [P, 64], DT.float32, kind="ExternalInput")
    idi = nc.dram_tensor("idi", [P, P], DT.float32, kind="ExternalInput")
    pool = nc.dram_tensor("pool", [ng, 12], DT.float32, kind="ExternalOutput")

    aspace = "Shared" if (use_cc and ncores > 4) else "Local"
    t1s = nc.dram_tensor("t1s", [shard, 72], DT.bfloat16, kind="Internal")
    t1f = nc.dram_tensor("t1f", [nt, 72], DT.bfloat16, kind="Internal",
                         addr_space=aspace)
    t2s = nc.dram_tensor("t2s", [nband * P, 16], DT.bfloat16, kind="Internal")
    t2f = nc.dram_tensor("t2f", [nt, 16], DT.bfloat16, kind="Internal",
                         addr_space=aspace)
    cc_groups = [list(range(ncores))]

    with tile.TileContext(nc) as tc:
        with (
            tc.tile_pool(name="cp", bufs=1) as cp,
            tc.tile_pool(name="sb", bufs=3) as sb,
            tc.tile_pool(name="pp", bufs=2, space="PSUM") as pp,
            tc.tile_pool(name="pq", bufs=1, space="PSUM") as pq,
        ):
            with contextlib.suppress(_PhaseStop):
                # ---- consts
                wfs = cp.tile([P, 80], DT.bfloat16)
                nc.sync.dma_start(wfs[:], wf[:, :])
                w2cs = cp.tile([64, 16], DT.bfloat16)
                nc.sync.dma_start(w2cs[:], w2c[:, :])
                srcis = cp.tile([P, SL], DT.int32)
                nc.sync.dma_start(srcis[:], srci[:, :])
                blis = cp.tile([P, nband], DT.bfloat16)
                nc.sync.dma_start(blis[:], bli[:, :])
                iogs = cp.tile([P, ng], DT.bfloat16)
                nc.sync.dma_start(iogs[:], iog[:, :])
                b1s = cp.tile([P, 64], DT.float32)
                nc.sync.dma_start(b1s[:], b1i[:, :])
                ids = cp.tile([P, P], DT.float32)
                nc.sync.dma_start(ids[:], idi[:, :])

                # ---- phase A: T1 shard build, node-major via stationary xT
                if "A" in phases:
                    xall = cp.tile([P, shard], DT.bfloat16)
                    nc.sync.dma_start(xall[:], xts[:, :])
                    nch = shard // P          # node chunks of 128
                    QB = 4                    # chunks per psum bank / write
                    for q0 in range(0, nch, QB):
                        qn = min(QB, nch - q0)
                        psA = pp.tile([P, QB, 80], DT.float32, tag="psA")
                        for qi in range(qn):
                            c0 = (q0 + qi) * P
                            nc.tensor.matmul(
                                out=psA[:, qi, :],
                                lhsT=xall[:, c0:c0 + P], rhs=wfs[:],
                                start=True, stop=True)
                        tb = sb.tile([P, QB, 72], DT.bfloat16, tag="tb")
                        nc.vector.tensor_copy(tb[:, 0:qn, :],
                                              psA[:, 0:qn, 0:72])
                        nc.sync.dma_start(
                            t1s[q0 * P:(q0 + qn) * P, :]
                            .rearrange("(q p) r -> p q r", q=qn),
                            tb[:, 0:qn, :])

                # ---- phase B: patch pad slot in t1s, AllGather T1
                if "B" not in phases:
                    raise _PhaseStop
                prt = cp.tile([1, 4], DT.bfloat16)
                nc.vector.memset(prt[:], -100.0)
                nc.sync.dma_start(t1s[nband * P - 1:nband * P, 0:4], prt[:])
                if use_cc:
                    nc.gpsimd.collective_compute(
                        "AllGather", OP.bypass, replica_groups=cc_groups,
                        ins=[t1s[:, :]], outs=[t1f[:, :]])
                else:
                    for i in range(ncores):
                        nc.sync.dma_start(t1f[i * shard:(i + 1) * shard, :],
                                          t1s[:, :])
                # ---- phase C: layer-1 edge pipeline per super-group
                if "C" not in phases:
                    raise _PhaseStop
                sd1 = cp.tile([P, nband, 72], DT.bfloat16)
                nc.sync.dma_start(
                    sd1[:], t1s[:, :].rearrange("(t p) r -> p t r", p=P))
                sd1f = cp.tile([P, nband, 4], DT.float32)
                nc.vector.tensor_copy(sd1f[:], sd1[:, :, 4:8])
                t2all = cp.tile([P, nband, 16], DT.bfloat16)
                nc.vector.memset(t2all[:], 0.0)

                cmax = int(os.environ.get("KV_CMAX", "9999"))
                for (t0, sg, lg) in groups[:cmax]:
                    S = sg * lg
                    o0 = off[t0]
                    g = sb.tile([P, sg, lg, 72], DT.bfloat16, tag="g1")
                    gf = g[:].rearrange("p b l r -> p (b l) r")
                    for b in range(sg):
                        lt = Ltrue[t0 + b]
                        if lt < lg:
                            nc.vector.memset(g[:, b, lt:lg, :], -100.0)
                        for j in range(lt):
                            jj = b * lg + j
                            nc.gpsimd.indirect_dma_start(
                                out=gf[:, jj, :], out_offset=None,
                                in_=t1f[:, :],
                                in_offset=bass.IndirectOffsetOnAxis(
                                    ap=srcis[:, o0 + jj:o0 + jj + 1], axis=0))
                    et = sb.tile([P, sg, 4, lg], DT.float32, tag="et")
                    nc.vector.tensor_tensor(
                        out=et[:],
                        in0=g[:, :, :, 0:4].rearrange("p b l h -> p b h l"),
                        in1=sd1f[:, t0:t0 + sg, :, None]
                            .to_broadcast([P, sg, 4, lg]),
                        op=OP.add)
                    e1 = sb.tile([P, sg, 4, lg], DT.float32, tag="e1")
                    nc.scalar.activation(
                        e1[:].rearrange("p b h l -> p (b h l)"),
                        et[:].rearrange("p b h l -> p (b h l)"), AF.Exp)
                    e2 = sb.tile([P, sg, 4, lg], DT.float32, tag="e2")
                    nc.scalar.activation(
                        e2[:].rearrange("p b h l -> p (b h l)"),
                        et[:].rearrange("p b h l -> p (b h l)"), AF.Exp,
                        scale=NEG)
                    p = sb.tile([P, sg, 4, lg], DT.bfloat16, tag="p1")
                    nc.vector.tensor_tensor(out=p[:], in0=e1[:], in1=e2[:],
                                            op=OP.max)
                    m = sb.tile([P, sg, 4, 16, lg], DT.bfloat16, tag="m1")
                    nc.vector.tensor_tensor(
                        out=m[:],
                        in0=g[:, :, :, 8:72]
                            .rearrange("p b l (h c) -> p b h c l", c=16),
                        in1=p[:, :, :, None, :]
                            .to_broadcast([P, sg, 4, 16, lg]),
                        op=OP.mult)
                    u = sb.tile([P, sg, 4, 16], DT.float32, tag="u1")
                    nc.vector.tensor_reduce(u[:], m[:], axis=AX.X, op=OP.add)
                    d = sb.tile([P, sg, 4], DT.float32, tag="d1")
                    nc.vector.tensor_reduce(d[:], p[:], axis=AX.X, op=OP.add)
                    nc.vector.tensor_scalar_add(d[:], d[:], 1e-16)
                    r = sb.tile([P, sg, 4], DT.float32, tag="r1")
                    nc.vector.reciprocal(r[:], d[:])
                    o = sb.tile([P, sg, 64], DT.float32, tag="o1")
                    nc.vector.tensor_tensor(
                        out=o[:].rearrange("p b (h c) -> p b h c", c=16),
                        in0=u[:],
                        in1=r[:, :, :, None].to_broadcast([P, sg, 4, 16]),
                        op=OP.mult)
                    if has_b1:
                        nc.vector.tensor_tensor(
                            out=o[:], in0=o[:],
                            in1=b1s[:, None, :].to_broadcast([P, sg, 64]),
                            op=OP.add)
                    xm = sb.tile([P, sg, 64], DT.float32, tag="xm")
                    nc.vector.tensor_scalar_min(xm[:], o[:], 0.0)
                    xe = sb.tile([P, sg, 64], DT.float32, tag="xe")
                    nc.scalar.activation(
                        xe[:].rearrange("p b c -> p (b c)"),
                        xm[:].rearrange("p b c -> p (b c)"), AF.Exp)
                    xr = sb.tile([P, sg, 64], DT.float32, tag="xr")
                    nc.scalar.activation(
                        xr[:].rearrange("p b c -> p (b c)"),
                        o[:].rearrange("p b c -> p (b c)"), AF.Relu)
                    o1 = sb.tile([P, sg, 64], DT.float32, tag="o1f")
                    nc.vector.tensor_tensor(out=o1[:], in0=xe[:], in1=xr[:],
                                            op=OP.add)
                    # T2 rows: transpose each band's [128, 64], matmul w2c
                    t2p = pp.tile([P, SG_MAX, 16], DT.float32, tag="t2p")
                    for b0 in range(0, sg, 4):
                        bn = min(4, sg - b0)
                        pst = pp.tile([64, 4, P], DT.float32, tag="pst")
                        for bi in range(bn):
                            nc.tensor.transpose(out=pst[:, bi, :],
                                                in_=o1[:, b0 + bi, :],
                                                identity=ids[:])
                        o1t = sb.tile([64, 4, P], DT.bfloat16, tag="o1t")
                        nc.vector.tensor_scalar_add(o1t[:, 0:bn, :],
                                                    pst[:, 0:bn, :], -1.0)
                        for bi in range(bn):
                            nc.tensor.matmul(
                                out=t2p[:, b0 + bi, :],
                                lhsT=o1t[:, bi, :], rhs=w2cs[:],
                                start=True, stop=True)
                    nc.vector.tensor_copy(t2all[:, t0:t0 + sg, :],
                                          t2p[:, 0:sg, :])

                # slot-ordered write of the whole T2 shard, then patch the
                # pad slot (last dummy slot): s2 cols = -100
                nc.sync.dma_start(
                    t2s[:, :].rearrange("(t p) r -> p t r", p=P), t2all[:])
                pr2 = cp.tile([1, 2], DT.bfloat16)
                nc.vector.memset(pr2[:], -100.0)
                nc.sync.dma_start(t2s[nband * P - 1:nband * P, 0:2], pr2[:])

                # ---- phase D: AllGather T2 + pad/tail patch
                if "D" not in phases:
                    raise _PhaseStop
                if use_cc:
                    nc.gpsimd.collective_compute(
                        "AllGather", OP.bypass, replica_groups=cc_groups,
                        ins=[t2s[:, :]], outs=[t2f[:, :]])
                else:
                    for i in range(ncores):
                        nc.sync.dma_start(
                            t2f[i * nband * P:(i + 1) * nband * P, :],
                            t2s[:, :])

                # ---- phase E: layer-2 edge pipeline + pooling
                if "E" not in phases:
                    raise _PhaseStop
                sd2f = cp.tile([P, nband, 1], DT.float32)
                nc.vector.tensor_copy(sd2f[:], t2all[:, :, 1:2])
                pps = pq.tile([ng, 12], DT.float32)

                for (t0, sg, lg) in groups:
                    S = sg * lg
                    o0 = off[t0]
                    g2 = sb.tile([P, sg, lg, 16], DT.bfloat16, tag="g2")
                    g2f = g2[:].rearrange("p b l r -> p (b l) r")
                    for b in range(sg):
                        lt = Ltrue[t0 + b]
                        if lt < lg:
                            nc.vector.memset(g2[:, b, lt:lg, :], -100.0)
                        for j in range(lt):
                            jj = b * lg + j
                            nc.gpsimd.indirect_dma_start(
                                out=g2f[:, jj, :], out_offset=None,
                                in_=t2f[:, :],
                                in_offset=bass.IndirectOffsetOnAxis(
                                    ap=srcis[:, o0 + jj:o0 + jj + 1], axis=0))
                    et2 = sb.tile([P, sg, lg], DT.float32, tag="et2")
                    nc.vector.tensor_tensor(
                        out=et2[:], in0=g2[:, :, :, 0],
                        in1=sd2f[:, t0:t0 + sg, :].to_broadcast([P, sg, lg]),
                        op=OP.add)
                    f1 = sb.tile([P, sg, lg], DT.float32, tag="f1")
                    nc.scalar.activation(
                        f1[:].rearrange("p b l -> p (b l)"),
                        et2[:].rearrange("p b l -> p (b l)"), AF.Exp)
                    f2 = sb.tile([P, sg, lg], DT.float32, tag="f2")
                    nc.scalar.activation(
                        f2[:].rearrange("p b l -> p (b l)"),
                        et2[:].rearrange("p b l -> p (b l)"), AF.Exp,
                        scale=NEG)
                    p2 = sb.tile([P, sg, lg], DT.bfloat16, tag="p2")
                    nc.vector.tensor_tensor(out=p2[:], in0=f1[:], in1=f2[:],
                                            op=OP.max)
                    m2 = sb.tile([P, sg, 10, lg], DT.bfloat16, tag="m2")
                    nc.vector.tensor_tensor(
                        out=m2[:],
                        in0=g2[:, :, :, 2:12].rearrange("p b l c -> p b c l"),
                        in1=p2[:, :, None, :].to_broadcast([P, sg, 10, lg]),
                        op=OP.mult)
                    u2 = sb.tile([P, sg, 10], DT.float32, tag="u2")
                    nc.vector.tensor_reduce(u2[:], m2[:], axis=AX.X, op=OP.add)
                    d2 = sb.tile([P, sg], DT.float32, tag="d2")
                    nc.vector.tensor_reduce(d2[:], p2[:], axis=AX.X, op=OP.add)
                    nc.vector.tensor_scalar_add(d2[:], d2[:], 1e-16)
                    r2 = sb.tile([P, sg], DT.float32, tag="r2")
                    nc.vector.reciprocal(r2[:], d2[:])
                    rhsp = sb.tile([P, sg, 12], DT.bfloat16, tag="rhsp")
                    nc.vector.memset(rhsp[:, :, 10:11], 1.0)
                    nc.vector.memset(rhsp[:, :, 11:12], 0.0)
                    nc.vector.tensor_tensor(
                        out=rhsp[:, :, 0:10], in0=u2[:],
                        in1=r2[:, :, None].to_broadcast([P, sg, 10]),
                        op=OP.mult)
                    sbh = sb.tile([P, sg, ng], DT.bfloat16, tag="sbh")
                    nc.vector.tensor_tensor(
                        out=sbh[:],
                        in0=blis[:, t0:t0 + sg, None].to_broadcast([P, sg, ng]),
                        in1=iogs[:, None, :].to_broadcast([P, sg, ng]),
                        op=OP.is_equal)
                    for b in range(sg):
                        tg = t0 + b
                        nc.tensor.matmul(out=pps[:], lhsT=sbh[:, b, :],
                                         rhs=rhsp[:, b, :],
                                         start=(tg == 0),
                                         stop=(tg == nband - 1),
                                         tile_position=(0, 0))

                po = cp.tile([ng, 12], DT.float32)
                nc.vector.tensor_copy(po[:], pps[:])
                nc.sync.dma_start(pool[:, :], po[:])

    nc.compile()
    return nc


# ---------------------------------------------------------------- launcher
class Launcher:
    def __init__(self, nc, n_cores):
        install_neuronx_cc_hook()
        self.nc = nc
        self.n_cores = n_cores
        pname = nc.partition_id_tensor.name if nc.partition_id_tensor else None
        in_names, out_names, out_avals, zero_outs = [], [], [], []
        for alloc in nc.m.functions[0].allocations:
            if not isinstance(alloc, mybir.MemoryLocationSet):
                continue
            name = alloc.memorylocations[0].name
            if alloc.kind == "ExternalInput":
                if name != pname:
                    in_names.append(name)
            elif alloc.kind == "ExternalOutput":
                out_names.append(name)
                shape = tuple(alloc.tensor_shape)
                dtype = mybir.dt.np(alloc.dtype)
                out_avals.append(jax.core.ShapedArray(shape, dtype))
                zero_outs.append(np.zeros(shape, dtype))
        self.in_names, self.out_names = in_names, out_names
        self.out_avals, self.zero_outs = out_avals, zero_outs
        n_params, n_outs = len(in_names), len(out_avals)
        all_in = in_names + out_names + ([pname] if pname else [])

        def _body(*args):
            operands = list(args)
            if pname is not None:
                operands.append(partition_id_tensor())
            return tuple(_bass_exec_p.bind(
                *operands, out_avals=tuple(out_avals), in_names=tuple(all_in),
                out_names=tuple(out_names), lowering_input_output_aliases=(),
                sim_require_finite=True, sim_require_nnan=True, nc=nc))

        devices = jax.devices()[:n_cores]
        self.mesh = Mesh(np.asarray(devices), ("core",))
        specs_in = (PartitionSpec("core"),) * (n_params + n_outs)
        specs_out = (PartitionSpec("core"),) * n_outs
        self.fn = jax.jit(shard_map(_body, mesh=self.mesh, in_specs=specs_in,
                                    out_specs=specs_out, check_rep=False),
                          keep_unused=True)
        self.sharding = NamedSharding(self.mesh, PartitionSpec("core"))

    def put(self, arr_percore):
        a = np.ascontiguousarray(arr_percore)
        return jax.device_put(a.reshape(a.shape[0] * a.shape[1], *a.shape[2:]),
                              self.sharding)

    def __call__(self, named_args):
        args = [named_args[n] for n in self.in_names]
        for z in self.zero_outs:
            zz = np.zeros((self.n_cores * z.shape[0], *z.shape[1:]), z.dtype)
            args.append(jax.device_put(zz, self.sharding))
        outs = self.fn(*args)
        return dict(zip(self.out_names, outs))


# ---------------------------------------------------------------- host side
_CACHE = {}


def make_inputs(x, edge_index, batch, W1, a1s, a1d, b1, W2, a2s, a2d, cfg, ep):
    n, ncores, npc = cfg["n"], cfg["ncores"], cfg["npc"]
    nband, shard, ng = cfg["nband"], cfg["shard"], cfg["ngraph"]
    nt = shard * ncores
    wf, w2c = build_weights(W1, a1s, a1d, W2, a2s, a2d)

    # xts: per-core xT columns in degree-sorted slot order (dummies -> 0)
    xtp = np.zeros((P, n + 1), np.float32)
    xtp[:, :n] = np.asarray(x, np.float32).T
    xtp = xtp.astype(BF16)
    slot_node = ep["slot_node"]                       # [ncores, nslot]
    sidx = np.where(slot_node >= 0, slot_node, n)
    xts = np.stack([xtp[:, sidx[c]] for c in range(ncores)])

    batch = np.asarray(batch, np.int64)
    bl_flat = np.where(slot_node >= 0,
                       batch[np.maximum(slot_node, 0)], 200)
    bl = np.ascontiguousarray(
        bl_flat.reshape(ncores, nband, P).transpose(0, 2, 1)).astype(BF16)

    rep = lambda a: np.broadcast_to(a, (ncores, *a.shape)).copy()
    iog = np.broadcast_to(np.arange(ng, dtype=np.float32).astype(BF16),
                          (P, ng)).copy()
    b1b = np.broadcast_to(np.asarray(b1, np.float32), (P, 64)).copy()
    ident = np.eye(P, dtype=np.float32)

    return {
        "xts": xts,
        "wf": rep(wf.astype(BF16)),
        "w2c": rep(w2c.astype(BF16)),
        "srci": ep["srcidx"],
        "bli": bl,
        "iog": rep(iog),
        "b1i": rep(b1b),
        "idi": rep(ident),
    }


def finish(pool_parts, b2, ng):
    acc = pool_parts.astype(np.float64).sum(axis=0)
    sums = acc[:, :10]
    cnts = np.maximum(acc[:, 10], 1.0)
    pooled = (sums / cnts[:, None] + np.asarray(b2, np.float64)).astype(np.float32)
    m = pooled.max(axis=1, keepdims=True)
    z = pooled - m
    return (z - np.log(np.exp(z).sum(axis=1, keepdims=True))).astype(np.float32)


def kernel(x, edge_index, batch, W1, att_src1, att_dst1, b1,
           W2, att_src2, att_dst2, b2):
    cfg = FULL
    ep = prep_edges(edge_index, cfg)
    key = (tuple(ep["L"]), bool(np.any(np.asarray(b1))))
    if key not in _CACHE:
        nc = build_module(cfg, ep, use_cc=True,
                          has_b1=bool(np.any(np.asarray(b1))))
        _CACHE[key] = Launcher(nc, cfg["ncores"])
    lau = _CACHE[key]

    named = make_inputs(x, edge_index, batch, W1, att_src1, att_dst1, b1,
                        W2, att_src2, att_dst2, cfg, ep)
    named = {k: lau.put(v) for k, v in named.items()}
    outs = lau(named)
    pool = np.asarray(outs["pool"]).reshape(cfg["ncores"], cfg["ngraph"], 12)
    return finish(pool, b2, cfg["ngraph"])



# revision 15
# speedup vs baseline: 2.5296x; 2.5296x over previous
"""2-layer GAT + mean-pool + log_softmax on 8 TRN2 NeuronCores — single launch.

Design v2 (dst-sharded, src-QUARTERED band grids, bulk dma_gather):
  - T1 rows [s_src(4)|s_dst(4)|h(64)|pad] in 256B-pitch tables (canonical
    per-core degree-sorted slot order), AllGathered to a full table.
  - The global row space (100352) exceeds dma_gather's int16 index range,
    so edges are split into 4 SRC QUARTERS (25088 rows each).  Each
    (core, quarter) gets its own degree-sorted band grid: one dst per
    partition, that quarter's incoming edges along the free dim.  One
    dma_gather per super-group fetches [128, S, 128] source rows with
    int16 quarter-local indices (994ns SWDGE overhead amortized over
    S*128 descriptors instead of per-column indirect DMAs).
  - softmax numerator p = exp(lrelu(s_src+s_dst)) = exp(max(y, .2y));
    per-quarter partial sums u = sum p*h, d = sum p are dma_scatter_add-ed
    (CCE add, int16 idx) into a canonical-order DRAM accumulator; s_dst
    is delivered to quarter layouts by small dma_scatter_add + strided
    reads.  Normalize + ELU + T2 build run in canonical band order.
  - Layer 2 repeats the same grids on the T2 table; per-graph pooling is
    a one-hot matmul in PSUM accumulated over the 98 canonical bands.
  - Host: sum 8 partial pools, mean, +b2, log_softmax.
Pad slots index quarter row 12543 (s_src patched to -100, h=0) => p ~ e^-20.
"""
import contextlib
import os
import numpy as np
import ml_dtypes

import jax
from jax.sharding import Mesh, PartitionSpec, NamedSharding
from jax.experimental.shard_map import shard_map

import concourse.bass as bass
import concourse.bacc as bacc
import concourse.mybir as mybir
import concourse.tile as tile
from concourse.bass2jax import _bass_exec_p, install_neuronx_cc_hook, partition_id_tensor

DT = mybir.dt
AF = mybir.ActivationFunctionType
OP = mybir.AluOpType
AX = mybir.AxisListType
BF16 = ml_dtypes.bfloat16
P = 128
NEG = 0.2
SLOT_BUDGET = int(os.environ.get("KV_SLOTS", "56"))
SG_MAX = int(os.environ.get("KV_SGMAX", "8"))

FULL = dict(n=100000, ncores=8, npc=12500, nslot=12544, nband=98,
            ngraph=64, nq=4, qrows=25088)


class _PhaseStop(Exception):
    pass


def _wrap16(vals):
    """Position i of a SWDGE index list lives at wrapped[i%16, i//16]."""
    n = vals.shape[-1]
    w = np.ascontiguousarray(vals.reshape(n // 16, 16).T)
    return np.tile(w, (8, 1))          # replicate to 128 partitions


# ---------------------------------------------------------------- host prep
def prep_edges(edge_index, cfg):
    n, ncores, npc = cfg["n"], cfg["ncores"], cfg["npc"]
    nslot, nband, nq, qrows = cfg["nslot"], cfg["nband"], cfg["nq"], cfg["qrows"]
    src = np.asarray(edge_index[0], dtype=np.int64)
    dst = np.asarray(edge_index[1], dtype=np.int64)
    loop = np.arange(n, dtype=np.int64)
    src = np.concatenate([src, loop])
    dst = np.concatenate([dst, loop])

    core = dst // npc
    ldst = dst - core * npc
    scq = (src // npc) // 2            # src quarter (2 cores per quarter)

    # canonical per-core layout: total-degree sort
    deg = np.zeros((ncores, npc), dtype=np.int64)
    np.add.at(deg, (core, ldst), 1)
    canon_of = np.zeros((ncores, npc), dtype=np.int64)   # node -> slot
    slot_node = np.full((ncores, nslot), -1, dtype=np.int64)
    for c in range(ncores):
        order = np.argsort(-deg[c], kind="stable")
        canon_of[c, order] = np.arange(npc)
        slot_node[c, :npc] = order + c * npc
    # global canonical row of every src node
    src_core = src // npc
    gslot = src_core * nslot + canon_of[src_core, src - src_core * npc]

    # per-(core, quarter) degree and sort
    degq = np.zeros((ncores, nq, npc), dtype=np.int64)
    np.add.at(degq, (core, scq, ldst), 1)
    qof = np.zeros((ncores, nq, npc), dtype=np.int64)    # node -> quarter slot
    qnode = np.full((ncores, nq, nslot), -1, dtype=np.int64)
    degs_sorted = np.zeros((ncores, nq, nslot), dtype=np.int64)
    for c in range(ncores):
        for k in range(nq):
            order = np.argsort(-degq[c, k], kind="stable")
            qof[c, k, order] = np.arange(npc)
            qnode[c, k, :npc] = order
            degs_sorted[c, k, :npc] = degq[c, k, order]
    # common band widths: max over cores
    bandmax = degs_sorted.reshape(ncores, nq, nband, P).max(axis=3)  # [nc,nq,98]
    Lq = bandmax.max(axis=0)                                         # [nq, 98]

    groups = []     # per quarter: list of (o0, sg, lg, t0)
    offs = []       # per quarter: per-band column offset
    NBq, SLq = [], []
    for k in range(nq):
        L = Lq[k].copy()
        nb = int(np.max(np.nonzero(L)[0])) + 1 if L.any() else 0
        g = []
        t0 = 0
        while t0 < nb:
            lg = max(int(L[t0]), 1)
            sg = 1
            while (t0 + sg < nb and sg < SG_MAX
                   and (sg + 1) * lg <= SLOT_BUDGET):
                sg += 1
            L[t0:t0 + sg] = lg
            g.append((t0, sg, lg))
            t0 += sg
        off = np.concatenate([[0], np.cumsum(L[:nb])[:-1]]) if nb else np.array([])
        groups.append([(int(off[t0]), sg, lg, t0) for (t0, sg, lg) in g])
        offs.append(off.astype(np.int64))
        NBq.append(nb)
        SLq.append(int(L[:nb].sum()))
    SLtot, NBtot = sum(SLq), sum(NBq)
    qcol = np.concatenate([[0], np.cumsum(SLq)[:-1]]).astype(np.int64)
    qband = np.concatenate([[0], np.cumsum(NBq)[:-1]]).astype(np.int64)

    # edge-slot index grids, int16 quarter-local rows; pads -> row 12543
    PAD = nslot - 1
    idxe = np.zeros((ncores, P, 8 * SLtot), dtype=np.int16)
    reali = np.zeros((ncores, P, 8 * NBtot), dtype=np.int16)
    sdqsi = np.zeros((ncores, P, 8 * nband * nq), dtype=np.int16)
    for c in range(ncores):
        for k in range(nq):
            m = (core == c) & (scq == k)
            dl = ldst[m]
            gs = gslot[m] - k * qrows
            slot = qof[c, k, dl]
            band = slot // P
            part = slot % P
            # column within band: running index per (band, part) pair
            key = slot
            ordk = np.argsort(key, kind="stable")
            key_s = key[ordk]
            starts = np.flatnonzero(np.r_[True, key_s[1:] != key_s[:-1]])
            reps = np.diff(np.r_[starts, len(key_s)])
            run = np.arange(len(key_s)) - np.repeat(starts, reps)
            col = np.empty(len(key_s), dtype=np.int64)
            col[ordk] = offs[k][band[ordk]] + run
            grid = np.full((P, SLq[k]), PAD, dtype=np.int16)
            grid[part, col] = gs.astype(np.int16)
            idxe[c, :, 8 * qcol[k]:8 * (qcol[k] + SLq[k])] = _wrap16(
                np.ascontiguousarray(grid.T).reshape(-1))[:, :]
            # realign targets: quarter slot s=(b*128+p) -> canonical row
            nb = NBq[k]
            s_ids = np.arange(nb * P)
            qn = qnode[c, k, s_ids]
            tgt = np.where(qn >= 0, canon_of[c, np.maximum(qn, 0)], s_ids)
            reali[c, :, 8 * qband[k]:8 * (qband[k] + nb)] = _wrap16(
                tgt.astype(np.int16))
            # sdq scatter: canonical slot s -> quarter slot
            s_ids = np.arange(nslot)
            cn = slot_node[c, s_ids]
            tq = np.where(cn >= 0, qof[c, k, np.maximum(cn - c * npc, 0)], s_ids)
            sdqsi[c, :, 8 * nband * k:8 * nband * (k + 1)] = _wrap16(
                tq.astype(np.int16))

    return dict(groups=groups, SLq=SLq, NBq=NBq, qcol=qcol, qband=qband,
                SLtot=SLtot, NBtot=NBtot, idxe=idxe, reali=reali,
                sdqsi=sdqsi, slot_node=slot_node)


def build_weights(W1, a1s, a1d, W2, a2s, a2d):
    W1T = np.asarray(W1, np.float32).T          # [F_in, 64]
    fin = W1T.shape[0]
    wf = np.zeros((fin, 80), np.float32)
    for h in range(4):
        wf[:, h] = W1T[:, 16 * h:16 * (h + 1)] @ np.asarray(a1s, np.float32)[h]
        wf[:, 4 + h] = W1T[:, 16 * h:16 * (h + 1)] @ np.asarray(a1d, np.float32)[h]
    wf[:, 8:72] = W1T
    W2T = np.asarray(W2, np.float32).T          # [64, 10]
    w2c = np.zeros((64, 16), np.float32)
    w2c[:, 0] = W2T @ np.asarray(a2s, np.float32).reshape(-1)
    w2c[:, 1] = W2T @ np.asarray(a2d, np.float32).reshape(-1)
    w2c[:, 2:12] = W2T
    return wf, w2c


# ---------------------------------------------------------------- module
def build_module(cfg, ep, use_cc=True, has_b1=True, phases="ABCDE"):
    n, ncores, npc = cfg["n"], cfg["ncores"], cfg["npc"]
    nslot, nband, ng = cfg["nslot"], cfg["nband"], cfg["ngraph"]
    nq, qrows = cfg["nq"], cfg["qrows"]
    groups, SLtot, NBtot = ep["groups"], ep["SLtot"], ep["NBtot"]
    qcol, qband, NBq = ep["qcol"], ep["qband"], ep["NBq"]
    nt = nslot * ncores
    nc = bacc.Bacc("TRN2", target_bir_lowering=False,
                   num_devices=ncores if use_cc else 1, num_swdge_queues=2)

    xts = nc.dram_tensor("xts", [P, nslot], DT.bfloat16, kind="ExternalInput")
    wf = nc.dram_tensor("wf", [P, 80], DT.bfloat16, kind="ExternalInput")
    w2c = nc.dram_tensor("w2c", [64, 16], DT.bfloat16, kind="ExternalInput")
    idxe = nc.dram_tensor("idxe", [P, 8 * SLtot], DT.int16, kind="ExternalInput")
    reali = nc.dram_tensor("reali", [P, 8 * NBtot], DT.int16, kind="ExternalInput")
    sdqsi = nc.dram_tensor("sdqsi", [P, 8 * nband * nq], DT.int16,
                           kind="ExternalInput")
    bli = nc.dram_tensor("bli", [P, nband], DT.bfloat16, kind="ExternalInput")
    iog = nc.dram_tensor("iog", [P, ng], DT.bfloat16, kind="ExternalInput")
    b1i = nc.dram_tensor("b1i", [P, 64], DT.float32, kind="ExternalInput")
    idi = nc.dram_tensor("idi", [P, P], DT.float32, kind="ExternalInput")
    pool = nc.dram_tensor("pool", [ng, 12], DT.float32, kind="ExternalOutput")

    aspace = "Shared" if (use_cc and ncores > 4) else "Local"
    t1sp = nc.dram_tensor("t1sp", [nslot, P], DT.bfloat16, kind="Internal")
    t1fp = nc.dram_tensor("t1fp", [nt, P], DT.bfloat16, kind="Internal",
                          addr_space=aspace)
    t2sp = nc.dram_tensor("t2sp", [nslot, P], DT.bfloat16, kind="Internal")
    t2fp = nc.dram_tensor("t2fp", [nt, P], DT.bfloat16, kind="Internal",
                          addr_space=aspace)
    sdqt1 = nc.dram_tensor("sdqt1", [nslot, nq * P], DT.bfloat16, kind="Internal")
    sdqt2 = nc.dram_tensor("sdqt2", [nslot, nq * P], DT.bfloat16, kind="Internal")
    uacc = nc.dram_tensor("uacc", [nslot, P], DT.bfloat16, kind="Internal")
    u2acc = nc.dram_tensor("u2acc", [nslot, P], DT.bfloat16, kind="Internal")
    cc_groups = [list(range(ncores))]

    ZB = 14                     # zero-init band chunk
    with tile.TileContext(nc) as tc:
        with (
            tc.tile_pool(name="cp", bufs=1) as cp,
            tc.tile_pool(name="sb", bufs=3) as sb,
            tc.tile_pool(name="pp", bufs=2, space="PSUM") as pp,
            tc.tile_pool(name="pq", bufs=1, space="PSUM") as pq,
        ):
            with contextlib.suppress(_PhaseStop):
                # ---- consts
                wfs = cp.tile([P, 80], DT.bfloat16)
                nc.sync.dma_start(wfs[:], wf[:, :])
                w2cs = cp.tile([64, 16], DT.bfloat16)
                nc.sync.dma_start(w2cs[:], w2c[:, :])
                blis = cp.tile([P, nband], DT.bfloat16)
                nc.sync.dma_start(blis[:], bli[:, :])
                iogs = cp.tile([P, ng], DT.bfloat16)
                nc.sync.dma_start(iogs[:], iog[:, :])
                b1s = cp.tile([P, 64], DT.float32)
                nc.sync.dma_start(b1s[:], b1i[:, :])
                ids = cp.tile([P, P], DT.float32)
                nc.sync.dma_start(ids[:], idi[:, :])
                sdqis = cp.tile([P, 8 * nband * nq], DT.int16)
                nc.sync.dma_start(sdqis[:], sdqsi[:, :])
                zt = cp.tile([P, ZB, P], DT.bfloat16)
                nc.vector.memset(zt[:], 0.0)
                sdc = cp.tile([P, nband, 11], DT.bfloat16)
                sdc2 = cp.tile([P, nband, 11], DT.bfloat16)

                # zero inits: t1sp pad cols, full t2sp/uacc/u2acc, sdq tables
                for b0 in range(0, nband, ZB):
                    bn = min(ZB, nband - b0)
                    rows = slice(b0 * P, (b0 + bn) * P)
                    nc.sync.dma_start(
                        t1sp[rows, 72:P].rearrange("(t p) r -> p t r", p=P),
                        zt[:, 0:bn, 0:56])
                    for t in (t2sp, uacc, u2acc):
                        nc.sync.dma_start(
                            t[rows, :].rearrange("(t p) r -> p t r", p=P),
                            zt[:, 0:bn, :])
                    for k in range(nq):
                        nc.sync.dma_start(
                            sdqt1[rows, k * P:k * P + 11]
                            .rearrange("(t p) r -> p t r", p=P),
                            zt[:, 0:bn, 0:11])
                        nc.sync.dma_start(
                            sdqt2[rows, k * P:k * P + 11]
                            .rearrange("(t p) r -> p t r", p=P),
                            zt[:, 0:bn, 0:11])

                # ---- phase A: T1 build (canonical node-major, stationary xT)
                if "A" in phases:
                    nch = nslot // P
                    QB = 4
                    for q0 in range(0, nch, QB):
                        qn = min(QB, nch - q0)
                        xc = sb.tile([P, QB, P], DT.bfloat16, tag="xc")
                        nc.sync.dma_start(
                            xc[:, 0:qn, :].rearrange("p q c -> p (q c)"),
                            xts[:, q0 * P:(q0 + qn) * P])
                        psA = pp.tile([P, QB, 80], DT.float32, tag="psA")
                        for qi in range(qn):
                            nc.tensor.matmul(
                                out=psA[:, qi, :],
                                lhsT=xc[:, qi, :], rhs=wfs[:],
                                start=True, stop=True)
                        tb = sb.tile([P, QB, 72], DT.bfloat16, tag="tb")
                        nc.vector.tensor_copy(tb[:, 0:qn, :],
                                              psA[:, 0:qn, 0:72])
                        nc.vector.tensor_copy(sdc[:, q0:q0 + qn, :],
                                              tb[:, 0:qn, 4:15])
                        nc.sync.dma_start(
                            t1sp[q0 * P:(q0 + qn) * P, 0:72]
                            .rearrange("(q p) r -> p q r", q=qn),
                            tb[:, 0:qn, :])

                # patch pad slot scores: s_src = -100
                if "B" not in phases:
                    raise _PhaseStop
                prt = cp.tile([1, 4], DT.bfloat16)
                nc.vector.memset(prt[:], -100.0)
                nc.sync.dma_start(t1sp[nslot - 1:nslot, 0:4], prt[:])
                # ---- phase B: AllGather T1 + s_dst quarter tables
                if use_cc:
                    nc.gpsimd.collective_compute(
                        "AllGather", OP.bypass, replica_groups=cc_groups,
                        ins=[t1sp[:, :]], outs=[t1fp[:, :]])
                else:
                    for i in range(ncores):
                        nc.sync.dma_start(t1fp[i * nslot:(i + 1) * nslot, :],
                                          t1sp[:, :])
                # SWDGE ring fits ~1024 desc; scatter m2s ~= idx/8 -> <=24
                # bands (3072 idx) per instruction
                sdq = cp.tile([P, NBtot, 4], DT.bfloat16)
                for k in range(nq):
                    for b0 in range(0, nband, 24):
                        bn = min(24, nband - b0)
                        nc.gpsimd.dma_scatter_add(
                            out_ap=sdqt1[:, k * P:k * P + 11],
                            in_ap=sdc[:, b0:b0 + bn, :],
                            idxs_ap=sdqis[:, 8 * (nband * k + b0):
                                          8 * (nband * k + b0 + bn)],
                            num_idxs=bn * P, num_idxs_reg=bn * P,
                            elem_size=11, elem_step=nq * P, queue_num=1, single_packet=False)
                    nc.sync.dma_start(
                        sdq[:, qband[k]:qband[k] + NBq[k], :],
                        sdqt1[0:NBq[k] * P, k * P + 4:k * P + 8]
                        .rearrange("(t p) r -> p t r", p=P))

                # ---- phase C: layer-1 edge pipeline per (quarter, group)
                if "C" not in phases:
                    raise _PhaseStop
                QMAX = int(os.environ.get("KV_QMAX", "4"))
                CMAX = int(os.environ.get("KV_CMAX", "9999"))
                NOSCAT = os.environ.get("KV_NOSCAT", "0") == "1"
                gsrc1, gsrc2 = t1fp, t2fp
                if os.environ.get("KV_LOCALT", "0") == "1":
                    t1fl = nc.dram_tensor("t1fl", [nt, P], DT.bfloat16,
                                          kind="Internal")
                    nc.sync.dma_start(t1fl[:, :], t1fp[:, :])
                    gsrc1 = t1fl
                for k in range(min(nq, QMAX)):
                    for (o0, sg, lg, t0) in groups[k][:CMAX]:
                        S = sg * lg
                        it = sb.tile([P, 8 * S], DT.int16, tag="it")
                        nc.sync.dma_start(
                            it[:], idxe[:, 8 * (qcol[k] + o0):
                                        8 * (qcol[k] + o0 + S)])
                        g = sb.tile([P, sg, lg, P], DT.bfloat16, tag="g1")
                        nc.gpsimd.dma_gather(
                            out_ap=g[:].rearrange("p b l r -> p (b l) r"),
                            in_ap=gsrc1[k * qrows:(k + 1) * qrows, :],
                            idxs_ap=it[:], num_idxs=S * P, num_idxs_reg=S * P,
                            elem_size=P, queue_num=0, single_packet=False)
                        et = sb.tile([P, sg, 4, lg], DT.float32, tag="et")
                        nc.vector.tensor_tensor(
                            out=et[:],
                            in0=g[:, :, :, 0:4].rearrange("p b l h -> p b h l"),
                            in1=sdq[:, qband[k] + t0:qband[k] + t0 + sg,
                                    :, None].to_broadcast([P, sg, 4, lg]),
                            op=OP.add)
                        lr = sb.tile([P, sg, 4, lg], DT.float32, tag="lr")
                        nc.vector.scalar_tensor_tensor(
                            out=lr[:], in0=et[:], scalar=NEG, in1=et[:],
                            op0=OP.mult, op1=OP.max)
                        p = sb.tile([P, sg, 4, lg], DT.bfloat16, tag="p1")
                        nc.scalar.activation(
                            p[:].rearrange("p b h l -> p (b h l)"),
                            lr[:].rearrange("p b h l -> p (b h l)"), AF.Exp)
                        m = sb.tile([P, sg, 4, 16, lg], DT.bfloat16, tag="m1")
                        nc.vector.tensor_tensor(
                            out=m[:],
                            in0=g[:, :, :, 8:72]
                                .rearrange("p b l (h c) -> p b h c l", c=16),
                            in1=p[:, :, :, None, :]
                                .to_broadcast([P, sg, 4, 16, lg]),
                            op=OP.mult)
                        urow = sb.tile([P, sg, 68], DT.bfloat16, tag="urow")
                        with nc.allow_low_precision(reason="bf16 partials"):
                            nc.vector.tensor_reduce(
                                urow[:, :, 0:64]
                                .rearrange("p b (h c) -> p b h c", c=16),
                                m[:], axis=AX.X, op=OP.add)
                            nc.vector.tensor_reduce(
                                urow[:, :, 64:68], p[:], axis=AX.X, op=OP.add)
                        if NOSCAT:
                            continue
                        rt = sb.tile([P, 8 * sg], DT.int16, tag="rt")
                        nc.sync.dma_start(
                            rt[:], reali[:, 8 * (qband[k] + t0):
                                         8 * (qband[k] + t0 + sg)])
                        nc.gpsimd.dma_scatter_add(
                            out_ap=uacc[:, 0:68], in_ap=urow[:],
                            idxs_ap=rt[:], num_idxs=sg * P,
                            num_idxs_reg=sg * P, elem_size=68,
                            elem_step=P, queue_num=1, single_packet=False)

                # ---- post-C: normalize + ELU + T2 build (canonical bands)
                if os.environ.get("KV_NOPOSTC", "0") == "1":
                    raise _PhaseStop
                CB = 8
                for b0 in range(0, nband, CB):
                    bn = min(CB, nband - b0)
                    un = sb.tile([P, CB, 68], DT.bfloat16, tag="un")
                    nc.sync.dma_start(
                        un[:, 0:bn, :],
                        uacc[b0 * P:(b0 + bn) * P, 0:68]
                        .rearrange("(t p) r -> p t r", p=P))
                    dn = sb.tile([P, CB, 4], DT.float32, tag="dn")
                    nc.vector.tensor_scalar_add(dn[:, 0:bn, :],
                                                un[:, 0:bn, 64:68], 1e-16)
                    r = sb.tile([P, CB, 4], DT.float32, tag="r1")
                    nc.vector.reciprocal(r[:, 0:bn, :], dn[:, 0:bn, :])
                    o = sb.tile([P, CB, 64], DT.float32, tag="o1")
                    nc.vector.tensor_tensor(
                        out=o[:, 0:bn, :].rearrange("p b (h c) -> p b h c",
                                                    c=16),
                        in0=un[:, 0:bn, 0:64]
                            .rearrange("p b (h c) -> p b h c", c=16),
                        in1=r[:, 0:bn, :, None].to_broadcast([P, bn, 4, 16]),
                        op=OP.mult)
                    if has_b1:
                        nc.vector.tensor_tensor(
                            out=o[:, 0:bn, :], in0=o[:, 0:bn, :],
                            in1=b1s[:, None, :].to_broadcast([P, bn, 64]),
                            op=OP.add)
                    xm = sb.tile([P, CB, 64], DT.float32, tag="xm")
                    nc.vector.tensor_scalar_min(xm[:, 0:bn, :], o[:, 0:bn, :],
                                                0.0)
                    xe = sb.tile([P, CB, 64], DT.float32, tag="xe")
                    nc.scalar.activation(
                        xe[:, 0:bn, :].rearrange("p b c -> p (b c)"),
                        xm[:, 0:bn, :].rearrange("p b c -> p (b c)"), AF.Exp)
                    xr = sb.tile([P, CB, 64], DT.float32, tag="xr")
                    nc.scalar.activation(
                        xr[:, 0:bn, :].rearrange("p b c -> p (b c)"),
                        o[:, 0:bn, :].rearrange("p b c -> p (b c)"), AF.Relu)
                    o1 = sb.tile([P, CB, 64], DT.float32, tag="o1f")
                    nc.vector.tensor_tensor(out=o1[:, 0:bn, :],
                                            in0=xe[:, 0:bn, :],
                                            in1=xr[:, 0:bn, :], op=OP.add)
                    # T2 rows: transpose each band's [128, 64], matmul w2c
                    t2p = pp.tile([P, CB, 16], DT.float32, tag="t2p")
                    for c0 in range(0, bn, 4):
                        cn = min(4, bn - c0)
                        pst = pp.tile([64, 4, P], DT.float32, tag="pst")
                        for ci in range(cn):
                            nc.tensor.transpose(out=pst[:, ci, :],
                                                in_=o1[:, c0 + ci, :],
                                                identity=ids[:])
                        o1t = sb.tile([64, 4, P], DT.bfloat16, tag="o1t")
                        nc.vector.tensor_scalar_add(o1t[:, 0:cn, :],
                                                    pst[:, 0:cn, :], -1.0)
                        for ci in range(cn):
                            nc.tensor.matmul(
                                out=t2p[:, c0 + ci, :],
                                lhsT=o1t[:, ci, :], rhs=w2cs[:],
                                start=True, stop=True)
                    t2c = sb.tile([P, CB, 16], DT.bfloat16, tag="t2c")
                    nc.vector.tensor_copy(t2c[:, 0:bn, :], t2p[:, 0:bn, :])
                    nc.vector.tensor_copy(sdc2[:, b0:b0 + bn, :],
                                          t2c[:, 0:bn, 0:11])
                    nc.sync.dma_start(
                        t2sp[b0 * P:(b0 + bn) * P, 0:16]
                        .rearrange("(q p) r -> p q r", q=bn),
                        t2c[:, 0:bn, :])
                pr2 = cp.tile([1, 2], DT.bfloat16)
                nc.vector.memset(pr2[:], -100.0)
                nc.sync.dma_start(t2sp[nslot - 1:nslot, 0:2], pr2[:])

                # ---- phase D: AllGather T2 + s_dst2 quarter tables
                if "D" not in phases:
                    raise _PhaseStop
                if use_cc:
                    nc.gpsimd.collective_compute(
                        "AllGather", OP.bypass, replica_groups=cc_groups,
                        ins=[t2sp[:, :]], outs=[t2fp[:, :]])
                else:
                    for i in range(ncores):
                        nc.sync.dma_start(t2fp[i * nslot:(i + 1) * nslot, :],
                                          t2sp[:, :])
                sd2q = cp.tile([P, NBtot, 2], DT.bfloat16)
                for k in range(nq):
                    for b0 in range(0, nband, 24):
                        bn = min(24, nband - b0)
                        nc.gpsimd.dma_scatter_add(
                            out_ap=sdqt2[:, k * P:k * P + 11],
                            in_ap=sdc2[:, b0:b0 + bn, :],
                            idxs_ap=sdqis[:, 8 * (nband * k + b0):
                                          8 * (nband * k + b0 + bn)],
                            num_idxs=bn * P, num_idxs_reg=bn * P,
                            elem_size=11, elem_step=nq * P, queue_num=1, single_packet=False)
                    nc.sync.dma_start(
                        sd2q[:, qband[k]:qband[k] + NBq[k], :],
                        sdqt2[0:NBq[k] * P, k * P:k * P + 2]
                        .rearrange("(t p) r -> p t r", p=P))

                # ---- phase E: layer-2 edge pipeline
                if "E" not in phases:
                    raise _PhaseStop
                for k in range(min(nq, QMAX)):
                    for (o0, sg, lg, t0) in groups[k][:CMAX]:
                        S = sg * lg
                        it2 = sb.tile([P, 8 * S], DT.int16, tag="it")
                        nc.sync.dma_start(
                            it2[:], idxe[:, 8 * (qcol[k] + o0):
                                         8 * (qcol[k] + o0 + S)])
                        g2 = sb.tile([P, sg, lg, P], DT.bfloat16, tag="g1")
                        nc.gpsimd.dma_gather(
                            out_ap=g2[:].rearrange("p b l r -> p (b l) r"),
                            in_ap=gsrc2[k * qrows:(k + 1) * qrows, :],
                            idxs_ap=it2[:], num_idxs=S * P,
                            num_idxs_reg=S * P, elem_size=P, queue_num=0, single_packet=False)
                        et2 = sb.tile([P, sg, lg], DT.float32, tag="et")
                        nc.vector.tensor_tensor(
                            out=et2[:], in0=g2[:, :, :, 0],
                            in1=sd2q[:, qband[k] + t0:qband[k] + t0 + sg, 1:2]
                                .to_broadcast([P, sg, lg]),
                            op=OP.add)
                        lr2 = sb.tile([P, sg, lg], DT.float32, tag="lr")
                        nc.vector.scalar_tensor_tensor(
                            out=lr2[:], in0=et2[:], scalar=NEG, in1=et2[:],
                            op0=OP.mult, op1=OP.max)
                        p2 = sb.tile([P, sg, lg], DT.bfloat16, tag="p1")
                        nc.scalar.activation(
                            p2[:].rearrange("p b l -> p (b l)"),
                            lr2[:].rearrange("p b l -> p (b l)"), AF.Exp)
                        m2 = sb.tile([P, sg, 10, lg], DT.bfloat16, tag="m1")
                        nc.vector.tensor_tensor(
                            out=m2[:],
                            in0=g2[:, :, :, 2:12]
                                .rearrange("p b l c -> p b c l"),
                            in1=p2[:, :, None, :].to_broadcast([P, sg, 10, lg]),
                            op=OP.mult)
                        u2row = sb.tile([P, sg, 11], DT.bfloat16, tag="urow")
                        with nc.allow_low_precision(reason="bf16 partials"):
                            nc.vector.tensor_reduce(
                                u2row[:, :, 0:10], m2[:], axis=AX.X, op=OP.add)
                            nc.vector.tensor_reduce(
                                u2row[:, :, 10:11], p2[:], axis=AX.X, op=OP.add)
                        rt2 = sb.tile([P, 8 * sg], DT.int16, tag="rt")
                        nc.sync.dma_start(
                            rt2[:], reali[:, 8 * (qband[k] + t0):
                                          8 * (qband[k] + t0 + sg)])
                        nc.gpsimd.dma_scatter_add(
                            out_ap=u2acc[:, 0:11], in_ap=u2row[:],
                            idxs_ap=rt2[:], num_idxs=sg * P,
                            num_idxs_reg=sg * P, elem_size=11,
                            elem_step=P, queue_num=1, single_packet=False)

                # ---- phase F: normalize + per-graph one-hot pooling
                un2 = cp.tile([P, nband, 11], DT.bfloat16)
                nc.sync.dma_start(
                    un2[:], u2acc[:, 0:11].rearrange("(t p) r -> p t r", p=P))
                pps = pq.tile([ng, 12], DT.float32)
                FB = 14
                for b0 in range(0, nband, FB):
                    bn = min(FB, nband - b0)
                    d2 = sb.tile([P, FB, 1], DT.float32, tag="d2")
                    nc.vector.tensor_scalar_add(d2[:, 0:bn, :],
                                                un2[:, b0:b0 + bn, 10:11],
                                                1e-16)
                    r2 = sb.tile([P, FB, 1], DT.float32, tag="r2")
                    nc.vector.reciprocal(r2[:, 0:bn, :], d2[:, 0:bn, :])
                    rhsp = sb.tile([P, FB, 12], DT.bfloat16, tag="rhsp")
                    nc.vector.memset(rhsp[:, 0:bn, 10:11], 1.0)
                    nc.vector.memset(rhsp[:, 0:bn, 11:12], 0.0)
                    nc.vector.tensor_tensor(
                        out=rhsp[:, 0:bn, 0:10], in0=un2[:, b0:b0 + bn, 0:10],
                        in1=r2[:, 0:bn, :].to_broadcast([P, bn, 10]),
                        op=OP.mult)
                    sbh = sb.tile([P, FB, ng], DT.bfloat16, tag="sbh")
                    nc.vector.tensor_tensor(
                        out=sbh[:, 0:bn, :],
                        in0=blis[:, b0:b0 + bn, None].to_broadcast([P, bn, ng]),
                        in1=iogs[:, None, :].to_broadcast([P, bn, ng]),
                        op=OP.is_equal)
                    for b in range(bn):
                        tg = b0 + b
                        nc.tensor.matmul(out=pps[:], lhsT=sbh[:, b, :],
                                         rhs=rhsp[:, b, :],
                                         start=(tg == 0),
                                         stop=(tg == nband - 1),
                                         tile_position=(0, 0))

                po = cp.tile([ng, 12], DT.float32)
                nc.vector.tensor_copy(po[:], pps[:])
                nc.sync.dma_start(pool[:, :], po[:])

    nc.compile()
    return nc


# ---------------------------------------------------------------- launcher
class Launcher:
    def __init__(self, nc, n_cores):
        install_neuronx_cc_hook()
        self.nc = nc
        self.n_cores = n_cores
        pname = nc.partition_id_tensor.name if nc.partition_id_tensor else None
        in_names, out_names, out_avals, zero_outs = [], [], [], []
        for alloc in nc.m.functions[0].allocations:
            if not isinstance(alloc, mybir.MemoryLocationSet):
                continue
            name = alloc.memorylocations[0].name
            if alloc.kind == "ExternalInput":
                if name != pname:
                    in_names.append(name)
            elif alloc.kind == "ExternalOutput":
                out_names.append(name)
                shape = tuple(alloc.tensor_shape)
                dtype = mybir.dt.np(alloc.dtype)
                out_avals.append(jax.core.ShapedArray(shape, dtype))
                zero_outs.append(np.zeros(shape, dtype))
        self.in_names, self.out_names = in_names, out_names
        self.out_avals, self.zero_outs = out_avals, zero_outs
        n_params, n_outs = len(in_names), len(out_avals)
        all_in = in_names + out_names + ([pname] if pname else [])

        def _body(*args):
            operands = list(args)
            if pname is not None:
                operands.append(partition_id_tensor())
            return tuple(_bass_exec_p.bind(
                *operands, out_avals=tuple(out_avals), in_names=tuple(all_in),
                out_names=tuple(out_names), lowering_input_output_aliases=(),
                sim_require_finite=True, sim_require_nnan=True, nc=nc))

        devices = jax.devices()[:n_cores]
        self.mesh = Mesh(np.asarray(devices), ("core",))
        specs_in = (PartitionSpec("core"),) * (n_params + n_outs)
        specs_out = (PartitionSpec("core"),) * n_outs
        self.fn = jax.jit(shard_map(_body, mesh=self.mesh, in_specs=specs_in,
                                    out_specs=specs_out, check_rep=False),
                          keep_unused=True)
        self.sharding = NamedSharding(self.mesh, PartitionSpec("core"))

    def put(self, arr_percore):
        a = np.ascontiguousarray(arr_percore)
        return jax.device_put(a.reshape(a.shape[0] * a.shape[1], *a.shape[2:]),
                              self.sharding)

    def __call__(self, named_args):
        args = [named_args[n] for n in self.in_names]
        for z in self.zero_outs:
            zz = np.zeros((self.n_cores * z.shape[0], *z.shape[1:]), z.dtype)
            args.append(jax.device_put(zz, self.sharding))
        outs = self.fn(*args)
        return dict(zip(self.out_names, outs))


# ---------------------------------------------------------------- host side
_CACHE = {}


def make_inputs(x, edge_index, batch, W1, a1s, a1d, b1, W2, a2s, a2d, cfg, ep):
    n, ncores, npc = cfg["n"], cfg["ncores"], cfg["npc"]
    nslot, nband, ng = cfg["nslot"], cfg["nband"], cfg["ngraph"]
    wf, w2c = build_weights(W1, a1s, a1d, W2, a2s, a2d)

    # xts: per-core xT columns in canonical slot order (dummies -> 0)
    xtp = np.zeros((P, n + 1), np.float32)
    xtp[:, :n] = np.asarray(x, np.float32).T
    xtp = xtp.astype(BF16)
    slot_node = ep["slot_node"]                       # [ncores, nslot]
    sidx = np.where(slot_node >= 0, slot_node, n)
    xts = np.stack([xtp[:, sidx[c]] for c in range(ncores)])

    batch = np.asarray(batch, np.int64)
    bl_flat = np.where(slot_node >= 0,
                       batch[np.maximum(slot_node, 0)], 200)
    bl = np.ascontiguousarray(
        bl_flat.reshape(ncores, nband, P).transpose(0, 2, 1)).astype(BF16)

    rep = lambda a: np.broadcast_to(a, (ncores, *a.shape)).copy()
    iog = np.broadcast_to(np.arange(ng, dtype=np.float32).astype(BF16),
                          (P, ng)).copy()
    b1b = np.broadcast_to(np.asarray(b1, np.float32), (P, 64)).copy()
    ident = np.eye(P, dtype=np.float32)

    return {
        "xts": xts,
        "wf": rep(wf.astype(BF16)),
        "w2c": rep(w2c.astype(BF16)),
        "idxe": ep["idxe"],
        "reali": ep["reali"],
        "sdqsi": ep["sdqsi"],
        "bli": bl,
        "iog": rep(iog),
        "b1i": rep(b1b),
        "idi": rep(ident),
    }


def finish(pool_parts, b2, ng):
    acc = pool_parts.astype(np.float64).sum(axis=0)
    sums = acc[:, :10]
    cnts = np.maximum(acc[:, 10], 1.0)
    pooled = (sums / cnts[:, None] + np.asarray(b2, np.float64)).astype(np.float32)
    m = pooled.max(axis=1, keepdims=True)
    z = pooled - m
    return (z - np.log(np.exp(z).sum(axis=1, keepdims=True))).astype(np.float32)


def kernel(x, edge_index, batch, W1, att_src1, att_dst1, b1,
           W2, att_src2, att_dst2, b2):
    cfg = FULL
    ep = prep_edges(edge_index, cfg)
    key = (tuple(tuple(map(tuple, g)) for g in ep["groups"]),
           bool(np.any(np.asarray(b1))))
    if key not in _CACHE:
        nc = build_module(cfg, ep, use_cc=True,
                          has_b1=bool(np.any(np.asarray(b1))))
        _CACHE[key] = Launcher(nc, cfg["ncores"])
    lau = _CACHE[key]

    named = make_inputs(x, edge_index, batch, W1, att_src1, att_dst1, b1,
                        W2, att_src2, att_dst2, cfg, ep)
    named = {k: lau.put(v) for k, v in named.items()}
    outs = lau(named)
    pool = np.asarray(outs["pool"]).reshape(cfg["ncores"], cfg["ngraph"], 12)
    return finish(pool, b2, cfg["ngraph"])


# revision 18
# speedup vs baseline: 2.8593x; 1.1303x over previous
"""2-layer GAT + mean-pool + log_softmax on 8 TRN2 NeuronCores — single launch.

Design v2 (dst-sharded, src-QUARTERED band grids, bulk dma_gather):
  - T1 rows [s_src(4)|s_dst(4)|h(64)|pad] in 256B-pitch tables (canonical
    per-core degree-sorted slot order), AllGathered to a full table.
  - The global row space (100352) exceeds dma_gather's int16 index range,
    so edges are split into 4 SRC QUARTERS (25088 rows each).  Each
    (core, quarter) gets its own degree-sorted band grid: one dst per
    partition, that quarter's incoming edges along the free dim.  One
    dma_gather per super-group fetches [128, S, 128] source rows with
    int16 quarter-local indices (994ns SWDGE overhead amortized over
    S*128 descriptors instead of per-column indirect DMAs).
  - softmax numerator p = exp(lrelu(s_src+s_dst)) = exp(max(y, .2y));
    per-quarter partial sums u = sum p*h, d = sum p are dma_scatter_add-ed
    (CCE add, int16 idx) into a canonical-order DRAM accumulator; s_dst
    is delivered to quarter layouts by small dma_scatter_add + strided
    reads.  Normalize + ELU + T2 build run in canonical band order.
  - Layer 2 repeats the same grids on the T2 table; per-graph pooling is
    a one-hot matmul in PSUM accumulated over the 98 canonical bands.
  - Host: sum 8 partial pools, mean, +b2, log_softmax.
Pad slots index quarter row 12543 (s_src patched to -100, h=0) => p ~ e^-20.
"""
import contextlib
import os
import numpy as np
import ml_dtypes

import jax
from jax.sharding import Mesh, PartitionSpec, NamedSharding
from jax.experimental.shard_map import shard_map

import concourse.bass as bass
import concourse.bacc as bacc
import concourse.mybir as mybir
import concourse.tile as tile
from concourse.bass2jax import _bass_exec_p, install_neuronx_cc_hook, partition_id_tensor

DT = mybir.dt
AF = mybir.ActivationFunctionType
OP = mybir.AluOpType
AX = mybir.AxisListType
BF16 = ml_dtypes.bfloat16
P = 128
NEG = 0.2
SLOT_BUDGET = int(os.environ.get("KV_SLOTS", "56"))
SG_MAX = int(os.environ.get("KV_SGMAX", "16"))

FULL = dict(n=100000, ncores=8, npc=12500, nslot=12544, nband=98,
            ngraph=64, nq=4, qrows=25088)


class _PhaseStop(Exception):
    pass


def _wrap16(vals):
    """Position i of a SWDGE index list lives at wrapped[i%16, i//16]."""
    n = vals.shape[-1]
    w = np.ascontiguousarray(vals.reshape(n // 16, 16).T)
    return np.tile(w, (8, 1))          # replicate to 128 partitions


# ---------------------------------------------------------------- host prep
def prep_edges(edge_index, cfg):
    n, ncores, npc = cfg["n"], cfg["ncores"], cfg["npc"]
    nslot, nband, nq, qrows = cfg["nslot"], cfg["nband"], cfg["nq"], cfg["qrows"]
    src = np.asarray(edge_index[0], dtype=np.int64)
    dst = np.asarray(edge_index[1], dtype=np.int64)
    loop = np.arange(n, dtype=np.int64)
    src = np.concatenate([src, loop])
    dst = np.concatenate([dst, loop])

    core = dst // npc
    ldst = dst - core * npc
    scq = (src // npc) // 2            # src quarter (2 cores per quarter)

    # canonical per-core layout: total-degree sort
    deg = np.zeros((ncores, npc), dtype=np.int64)
    np.add.at(deg, (core, ldst), 1)
    canon_of = np.zeros((ncores, npc), dtype=np.int64)   # node -> slot
    slot_node = np.full((ncores, nslot), -1, dtype=np.int64)
    for c in range(ncores):
        order = np.argsort(-deg[c], kind="stable")
        canon_of[c, order] = np.arange(npc)
        slot_node[c, :npc] = order + c * npc
    # global canonical row of every src node
    src_core = src // npc
    gslot = src_core * nslot + canon_of[src_core, src - src_core * npc]

    # per-(core, quarter) degree and sort — self-loops excluded (their
    # contribution is computed in canonical space, no gather needed)
    ns = src != dst
    degq = np.zeros((ncores, nq, npc), dtype=np.int64)
    np.add.at(degq, (core[ns], scq[ns], ldst[ns]), 1)
    qof = np.zeros((ncores, nq, npc), dtype=np.int64)    # node -> quarter slot
    qnode = np.full((ncores, nq, nslot), -1, dtype=np.int64)
    degs_sorted = np.zeros((ncores, nq, nslot), dtype=np.int64)
    for c in range(ncores):
        for k in range(nq):
            order = np.argsort(-degq[c, k], kind="stable")
            qof[c, k, order] = np.arange(npc)
            qnode[c, k, :npc] = order
            degs_sorted[c, k, :npc] = degq[c, k, order]
    # common band widths: max over cores
    bandmax = degs_sorted.reshape(ncores, nq, nband, P).max(axis=3)  # [nc,nq,98]
    Lq = bandmax.max(axis=0)                                         # [nq, 98]

    groups = []     # per quarter: list of (o0, sg, lg, t0)
    offs = []       # per quarter: per-band column offset
    NBq, SLq = [], []
    for k in range(nq):
        L = Lq[k].copy()
        nb = int(np.max(np.nonzero(L)[0])) + 1 if L.any() else 0
        g = []
        t0 = 0
        while t0 < nb:
            lg = max(int(L[t0]), 1)
            sg = 1
            while (t0 + sg < nb and sg < SG_MAX
                   and (sg + 1) * lg <= SLOT_BUDGET
                   and lg - L[t0 + sg] <= max(1, lg // 8)):
                sg += 1
            L[t0:t0 + sg] = lg
            g.append((t0, sg, lg))
            t0 += sg
        off = np.concatenate([[0], np.cumsum(L[:nb])[:-1]]) if nb else np.array([])
        groups.append([(int(off[t0]), sg, lg, t0) for (t0, sg, lg) in g])
        offs.append(off.astype(np.int64))
        NBq.append(nb)
        SLq.append(int(L[:nb].sum()))
    SLtot, NBtot = sum(SLq), sum(NBq)
    qcol = np.concatenate([[0], np.cumsum(SLq)[:-1]]).astype(np.int64)
    qband = np.concatenate([[0], np.cumsum(NBq)[:-1]]).astype(np.int64)

    # edge-slot index grids, int16 quarter-local rows; pads -> row 12543
    PAD = nslot - 1
    idxe = np.zeros((ncores, P, 8 * SLtot), dtype=np.int16)
    reali = np.zeros((ncores, P, 8 * NBtot), dtype=np.int16)
    sdqsi = np.zeros((ncores, P, 8 * nband * nq), dtype=np.int16)
    for c in range(ncores):
        for k in range(nq):
            m = (core == c) & (scq == k) & ns
            dl = ldst[m]
            gs = gslot[m] - k * qrows
            slot = qof[c, k, dl]
            band = slot // P
            part = slot % P
            # column within band: running index per (band, part) pair
            key = slot
            ordk = np.argsort(key, kind="stable")
            key_s = key[ordk]
            starts = np.flatnonzero(np.r_[True, key_s[1:] != key_s[:-1]])
            reps = np.diff(np.r_[starts, len(key_s)])
            run = np.arange(len(key_s)) - np.repeat(starts, reps)
            col = np.empty(len(key_s), dtype=np.int64)
            col[ordk] = offs[k][band[ordk]] + run
            grid = np.full((P, SLq[k]), PAD, dtype=np.int16)
            grid[part, col] = gs.astype(np.int16)
            idxe[c, :, 8 * qcol[k]:8 * (qcol[k] + SLq[k])] = _wrap16(
                np.ascontiguousarray(grid.T).reshape(-1))[:, :]
            # realign targets: quarter slot s=(b*128+p) -> canonical row
            nb = NBq[k]
            s_ids = np.arange(nb * P)
            qn = qnode[c, k, s_ids]
            tgt = np.where(qn >= 0, canon_of[c, np.maximum(qn, 0)], s_ids)
            reali[c, :, 8 * qband[k]:8 * (qband[k] + nb)] = _wrap16(
                tgt.astype(np.int16))
            # sdq scatter: canonical slot s -> quarter slot
            s_ids = np.arange(nslot)
            cn = slot_node[c, s_ids]
            tq = np.where(cn >= 0, qof[c, k, np.maximum(cn - c * npc, 0)], s_ids)
            sdqsi[c, :, 8 * nband * k:8 * nband * (k + 1)] = _wrap16(
                tq.astype(np.int16))

    selfi = np.broadcast_to(_wrap16(np.arange(nslot, dtype=np.int16)),
                            (ncores, P, 8 * nband)).copy()
    return dict(groups=groups, SLq=SLq, NBq=NBq, qcol=qcol, qband=qband,
                SLtot=SLtot, NBtot=NBtot, idxe=idxe, reali=reali,
                sdqsi=sdqsi, selfi=selfi, slot_node=slot_node)


def build_weights(W1, a1s, a1d, W2, a2s, a2d):
    W1T = np.asarray(W1, np.float32).T          # [F_in, 64]
    fin = W1T.shape[0]
    wf = np.zeros((fin, 80), np.float32)
    for h in range(4):
        wf[:, h] = W1T[:, 16 * h:16 * (h + 1)] @ np.asarray(a1s, np.float32)[h]
        wf[:, 4 + h] = W1T[:, 16 * h:16 * (h + 1)] @ np.asarray(a1d, np.float32)[h]
    wf[:, 8:72] = W1T
    W2T = np.asarray(W2, np.float32).T          # [64, 10]
    w2c = np.zeros((64, 16), np.float32)
    w2c[:, 0] = W2T @ np.asarray(a2s, np.float32).reshape(-1)
    w2c[:, 1] = W2T @ np.asarray(a2d, np.float32).reshape(-1)
    w2c[:, 2:12] = W2T
    return wf, w2c


# ---------------------------------------------------------------- module
def build_module(cfg, ep, use_cc=True, has_b1=True, phases="ABCDE"):
    n, ncores, npc = cfg["n"], cfg["ncores"], cfg["npc"]
    nslot, nband, ng = cfg["nslot"], cfg["nband"], cfg["ngraph"]
    nq, qrows = cfg["nq"], cfg["qrows"]
    groups, SLtot, NBtot = ep["groups"], ep["SLtot"], ep["NBtot"]
    qcol, qband, NBq = ep["qcol"], ep["qband"], ep["NBq"]
    nt = nslot * ncores
    nc = bacc.Bacc("TRN2", target_bir_lowering=False,
                   num_devices=ncores if use_cc else 1, num_swdge_queues=2)

    xts = nc.dram_tensor("xts", [P, nslot], DT.bfloat16, kind="ExternalInput")
    wf = nc.dram_tensor("wf", [P, 80], DT.bfloat16, kind="ExternalInput")
    w2c = nc.dram_tensor("w2c", [64, 16], DT.bfloat16, kind="ExternalInput")
    idxe = nc.dram_tensor("idxe", [P, 8 * SLtot], DT.int16, kind="ExternalInput")
    reali = nc.dram_tensor("reali", [P, 8 * NBtot], DT.int16, kind="ExternalInput")
    sdqsi = nc.dram_tensor("sdqsi", [P, 8 * nband * nq], DT.int16,
                           kind="ExternalInput")
    selfi = nc.dram_tensor("selfi", [P, 8 * nband], DT.int16,
                           kind="ExternalInput")
    bli = nc.dram_tensor("bli", [P, nband], DT.bfloat16, kind="ExternalInput")
    iog = nc.dram_tensor("iog", [P, ng], DT.bfloat16, kind="ExternalInput")
    b1i = nc.dram_tensor("b1i", [P, 64], DT.float32, kind="ExternalInput")
    idi = nc.dram_tensor("idi", [P, P], DT.float32, kind="ExternalInput")
    pool = nc.dram_tensor("pool", [ng, 12], DT.float32, kind="ExternalOutput")

    aspace = "Shared" if (use_cc and ncores > 4) else "Local"
    t1sp = nc.dram_tensor("t1sp", [nslot, P], DT.bfloat16, kind="Internal")
    t1fp = nc.dram_tensor("t1fp", [nt, P], DT.bfloat16, kind="Internal",
                          addr_space=aspace)
    t2sp = nc.dram_tensor("t2sp", [nslot, P], DT.bfloat16, kind="Internal")
    t2fp = nc.dram_tensor("t2fp", [nt, P], DT.bfloat16, kind="Internal",
                          addr_space=aspace)
    sdqt1 = nc.dram_tensor("sdqt1", [nslot, nq * P], DT.bfloat16, kind="Internal")
    sdqt2 = nc.dram_tensor("sdqt2", [nslot, nq * P], DT.bfloat16, kind="Internal")
    uacc = nc.dram_tensor("uacc", [nslot, P], DT.bfloat16, kind="Internal")
    u2acc = nc.dram_tensor("u2acc", [nslot, P], DT.bfloat16, kind="Internal")
    cc_groups = [list(range(ncores))]

    ZB = 14                     # zero-init band chunk
    with tile.TileContext(nc) as tc:
        with (
            tc.tile_pool(name="cp", bufs=1) as cp,
            tc.tile_pool(name="sb", bufs=3) as sb,
            tc.tile_pool(name="pp", bufs=2, space="PSUM") as pp,
            tc.tile_pool(name="pq", bufs=1, space="PSUM") as pq,
        ):
            with contextlib.suppress(_PhaseStop):
                # ---- consts
                wfs = cp.tile([P, 80], DT.bfloat16)
                nc.sync.dma_start(wfs[:], wf[:, :])
                w2cs = cp.tile([64, 16], DT.bfloat16)
                nc.sync.dma_start(w2cs[:], w2c[:, :])
                blis = cp.tile([P, nband], DT.bfloat16)
                nc.sync.dma_start(blis[:], bli[:, :])
                iogs = cp.tile([P, ng], DT.bfloat16)
                nc.sync.dma_start(iogs[:], iog[:, :])
                b1s = cp.tile([P, 64], DT.float32)
                nc.sync.dma_start(b1s[:], b1i[:, :])
                ids = cp.tile([P, P], DT.float32)
                nc.sync.dma_start(ids[:], idi[:, :])
                sdqis = cp.tile([P, 8 * nband * nq], DT.int16)
                nc.sync.dma_start(sdqis[:], sdqsi[:, :])
                zt = cp.tile([P, ZB, P], DT.bfloat16)
                nc.vector.memset(zt[:], 0.0)
                sdc = cp.tile([P, nband, 11], DT.bfloat16)
                sdc2 = cp.tile([P, nband, 11], DT.bfloat16)
                sfis = cp.tile([P, 8 * nband], DT.int16)
                nc.sync.dma_start(sfis[:], selfi[:, :])
                uself = cp.tile([P, nband, 68], DT.bfloat16)

                # zero inits: t1sp pad cols, full t2sp/uacc/u2acc, sdq tables
                for b0 in range(0, nband, ZB):
                    bn = min(ZB, nband - b0)
                    rows = slice(b0 * P, (b0 + bn) * P)
                    nc.sync.dma_start(
                        t1sp[rows, 72:P].rearrange("(t p) r -> p t r", p=P),
                        zt[:, 0:bn, 0:56])
                    for t in (t2sp, uacc, u2acc):
                        nc.sync.dma_start(
                            t[rows, :].rearrange("(t p) r -> p t r", p=P),
                            zt[:, 0:bn, :])
                    for k in range(nq):
                        nc.sync.dma_start(
                            sdqt1[rows, k * P:k * P + 11]
                            .rearrange("(t p) r -> p t r", p=P),
                            zt[:, 0:bn, 0:11])
                        nc.sync.dma_start(
                            sdqt2[rows, k * P:k * P + 11]
                            .rearrange("(t p) r -> p t r", p=P),
                            zt[:, 0:bn, 0:11])

                # ---- phase A: T1 build (canonical node-major, stationary xT)
                if "A" in phases:
                    nch = nslot // P
                    QB = 4
                    for q0 in range(0, nch, QB):
                        qn = min(QB, nch - q0)
                        xc = sb.tile([P, QB, P], DT.bfloat16, tag="xc")
                        nc.sync.dma_start(
                            xc[:, 0:qn, :].rearrange("p q c -> p (q c)"),
                            xts[:, q0 * P:(q0 + qn) * P])
                        psA = pp.tile([P, QB, 80], DT.float32, tag="psA")
                        for qi in range(qn):
                            nc.tensor.matmul(
                                out=psA[:, qi, :],
                                lhsT=xc[:, qi, :], rhs=wfs[:],
                                start=True, stop=True)
                        tb = sb.tile([P, QB, 72], DT.bfloat16, tag="tb")
                        nc.vector.tensor_copy(tb[:, 0:qn, :],
                                              psA[:, 0:qn, 0:72])
                        nc.vector.tensor_copy(sdc[:, q0:q0 + qn, :],
                                              tb[:, 0:qn, 4:15])
                        ys = sb.tile([P, QB, 4], DT.float32, tag="ys")
                        nc.vector.tensor_tensor(
                            out=ys[:, 0:qn, :], in0=tb[:, 0:qn, 0:4],
                            in1=tb[:, 0:qn, 4:8], op=OP.add)
                        nc.vector.scalar_tensor_tensor(
                            out=ys[:, 0:qn, :], in0=ys[:, 0:qn, :],
                            scalar=NEG, in1=ys[:, 0:qn, :],
                            op0=OP.mult, op1=OP.max)
                        nc.scalar.activation(
                            uself[:, q0:q0 + qn, 64:68],
                            ys[:, 0:qn, :], AF.Exp)
                        nc.vector.tensor_tensor(
                            out=uself[:, q0:q0 + qn, 0:64]
                            .rearrange("p q (h c) -> p q h c", c=16),
                            in0=tb[:, 0:qn, 8:72]
                            .rearrange("p q (h c) -> p q h c", c=16),
                            in1=uself[:, q0:q0 + qn, 64:68][:, :, :, None]
                            .to_broadcast([P, qn, 4, 16]),
                            op=OP.mult)
                        nc.sync.dma_start(
                            t1sp[q0 * P:(q0 + qn) * P, 0:72]
                            .rearrange("(q p) r -> p q r", q=qn),
                            tb[:, 0:qn, :])

                for b0 in range(0, nband, 24):
                    bn = min(24, nband - b0)
                    nc.gpsimd.dma_scatter_add(
                        out_ap=uacc[:, 0:68], in_ap=uself[:, b0:b0 + bn, :],
                        idxs_ap=sfis[:, 8 * b0:8 * (b0 + bn)],
                        num_idxs=bn * P, num_idxs_reg=bn * P, elem_size=68,
                        elem_step=P, queue_num=1, single_packet=False)

                # patch pad slot scores: s_src = -100
                if "B" not in phases:
                    raise _PhaseStop
                prt = cp.tile([1, 4], DT.bfloat16)
                nc.vector.memset(prt[:], -100.0)
                nc.sync.dma_start(t1sp[nslot - 1:nslot, 0:4], prt[:])
                # ---- phase B: AllGather T1 + s_dst quarter tables
                if use_cc:
                    nc.gpsimd.collective_compute(
                        "AllGather", OP.bypass, replica_groups=cc_groups,
                        ins=[t1sp[:, :]], outs=[t1fp[:, :]])
                else:
                    for i in range(ncores):
                        nc.sync.dma_start(t1fp[i * nslot:(i + 1) * nslot, :],
                                          t1sp[:, :])
                # SWDGE ring fits ~1024 desc; scatter m2s ~= idx/8 -> <=24
                # bands (3072 idx) per instruction
                sdq = cp.tile([P, NBtot, 4], DT.bfloat16)
                for k in range(nq):
                    for b0 in range(0, nband, 24):
                        bn = min(24, nband - b0)
                        nc.gpsimd.dma_scatter_add(
                            out_ap=sdqt1[:, k * P:k * P + 11],
                            in_ap=sdc[:, b0:b0 + bn, :],
                            idxs_ap=sdqis[:, 8 * (nband * k + b0):
                                          8 * (nband * k + b0 + bn)],
                            num_idxs=bn * P, num_idxs_reg=bn * P,
                            elem_size=11, elem_step=nq * P, queue_num=1, single_packet=False)
                    nc.sync.dma_start(
                        sdq[:, qband[k]:qband[k] + NBq[k], :],
                        sdqt1[0:NBq[k] * P, k * P + 4:k * P + 8]
                        .rearrange("(t p) r -> p t r", p=P))

                # ---- phase C: layer-1 edge pipeline per (quarter, group)
                if "C" not in phases:
                    raise _PhaseStop
                QMAX = int(os.environ.get("KV_QMAX", "4"))
                CMAX = int(os.environ.get("KV_CMAX", "9999"))
                NOSCAT = os.environ.get("KV_NOSCAT", "0") == "1"
                gsrc1, gsrc2 = t1fp, t2fp
                if os.environ.get("KV_LOCALT", "0") == "1":
                    t1fl = nc.dram_tensor("t1fl", [nt, P], DT.bfloat16,
                                          kind="Internal")
                    nc.sync.dma_start(t1fl[:, :], t1fp[:, :])
                    gsrc1 = t1fl
                for k in range(min(nq, QMAX)):
                    for (o0, sg, lg, t0) in groups[k][:CMAX]:
                        S = sg * lg
                        it = sb.tile([P, 8 * S], DT.int16, tag="it")
                        nc.sync.dma_start(
                            it[:], idxe[:, 8 * (qcol[k] + o0):
                                        8 * (qcol[k] + o0 + S)])
                        g = sb.tile([P, sg, lg, P], DT.bfloat16, tag="g1")
                        nc.gpsimd.dma_gather(
                            out_ap=g[:].rearrange("p b l r -> p (b l) r"),
                            in_ap=gsrc1[k * qrows:(k + 1) * qrows, :],
                            idxs_ap=it[:], num_idxs=S * P, num_idxs_reg=S * P,
                            elem_size=P, queue_num=0, single_packet=False)
                        et = sb.tile([P, sg, 4, lg], DT.float32, tag="et")
                        nc.vector.tensor_tensor(
                            out=et[:],
                            in0=g[:, :, :, 0:4].rearrange("p b l h -> p b h l"),
                            in1=sdq[:, qband[k] + t0:qband[k] + t0 + sg,
                                    :, None].to_broadcast([P, sg, 4, lg]),
                            op=OP.add)
                        lr = sb.tile([P, sg, 4, lg], DT.float32, tag="lr")
                        nc.vector.scalar_tensor_tensor(
                            out=lr[:], in0=et[:], scalar=NEG, in1=et[:],
                            op0=OP.mult, op1=OP.max)
                        p = sb.tile([P, sg, 4, lg], DT.bfloat16, tag="p1")
                        nc.scalar.activation(
                            p[:].rearrange("p b h l -> p (b h l)"),
                            lr[:].rearrange("p b h l -> p (b h l)"), AF.Exp)
                        m = sb.tile([P, sg, 4, 16, lg], DT.bfloat16, tag="m1")
                        nc.vector.tensor_tensor(
                            out=m[:],
                            in0=g[:, :, :, 8:72]
                                .rearrange("p b l (h c) -> p b h c l", c=16),
                            in1=p[:, :, :, None, :]
                                .to_broadcast([P, sg, 4, 16, lg]),
                            op=OP.mult)
                        urow = sb.tile([P, sg, 68], DT.bfloat16, tag="urow")
                        with nc.allow_low_precision(reason="bf16 partials"):
                            nc.vector.tensor_reduce(
                                urow[:, :, 0:64]
                                .rearrange("p b (h c) -> p b h c", c=16),
                                m[:], axis=AX.X, op=OP.add)
                            nc.vector.tensor_reduce(
                                urow[:, :, 64:68], p[:], axis=AX.X, op=OP.add)
                        if NOSCAT:
                            continue
                        rt = sb.tile([P, 8 * sg], DT.int16, tag="rt")
                        nc.sync.dma_start(
                            rt[:], reali[:, 8 * (qband[k] + t0):
                                         8 * (qband[k] + t0 + sg)])
                        nc.gpsimd.dma_scatter_add(
                            out_ap=uacc[:, 0:68], in_ap=urow[:],
                            idxs_ap=rt[:], num_idxs=sg * P,
                            num_idxs_reg=sg * P, elem_size=68,
                            elem_step=P, queue_num=1, single_packet=False)

                # ---- post-C: normalize + ELU + T2 build (canonical bands)
                if os.environ.get("KV_NOPOSTC", "0") == "1":
                    raise _PhaseStop
                CB = 8
                for b0 in range(0, nband, CB):
                    bn = min(CB, nband - b0)
                    un = sb.tile([P, CB, 68], DT.bfloat16, tag="un")
                    nc.sync.dma_start(
                        un[:, 0:bn, :],
                        uacc[b0 * P:(b0 + bn) * P, 0:68]
                        .rearrange("(t p) r -> p t r", p=P))
                    dn = sb.tile([P, CB, 4], DT.float32, tag="dn")
                    nc.vector.tensor_scalar_add(dn[:, 0:bn, :],
                                                un[:, 0:bn, 64:68], 1e-16)
                    r = sb.tile([P, CB, 4], DT.float32, tag="r1")
                    nc.vector.reciprocal(r[:, 0:bn, :], dn[:, 0:bn, :])
                    o = sb.tile([P, CB, 64], DT.float32, tag="o1")
                    nc.vector.tensor_tensor(
                        out=o[:, 0:bn, :].rearrange("p b (h c) -> p b h c",
                                                    c=16),
                        in0=un[:, 0:bn, 0:64]
                            .rearrange("p b (h c) -> p b h c", c=16),
                        in1=r[:, 0:bn, :, None].to_broadcast([P, bn, 4, 16]),
                        op=OP.mult)
                    if has_b1:
                        nc.vector.tensor_tensor(
                            out=o[:, 0:bn, :], in0=o[:, 0:bn, :],
                            in1=b1s[:, None, :].to_broadcast([P, bn, 64]),
                            op=OP.add)
                    xm = sb.tile([P, CB, 64], DT.float32, tag="xm")
                    nc.vector.tensor_scalar_min(xm[:, 0:bn, :], o[:, 0:bn, :],
                                                0.0)
                    xe = sb.tile([P, CB, 64], DT.float32, tag="xe")
                    nc.scalar.activation(
                        xe[:, 0:bn, :].rearrange("p b c -> p (b c)"),
                        xm[:, 0:bn, :].rearrange("p b c -> p (b c)"), AF.Exp)
                    xr = sb.tile([P, CB, 64], DT.float32, tag="xr")
                    nc.scalar.activation(
                        xr[:, 0:bn, :].rearrange("p b c -> p (b c)"),
                        o[:, 0:bn, :].rearrange("p b c -> p (b c)"), AF.Relu)
                    o1 = sb.tile([P, CB, 64], DT.float32, tag="o1f")
                    nc.vector.tensor_tensor(out=o1[:, 0:bn, :],
                                            in0=xe[:, 0:bn, :],
                                            in1=xr[:, 0:bn, :], op=OP.add)
                    # T2 rows: transpose each band's [128, 64], matmul w2c
                    t2p = pp.tile([P, CB, 16], DT.float32, tag="t2p")
                    for c0 in range(0, bn, 4):
                        cn = min(4, bn - c0)
                        pst = pp.tile([64, 4, P], DT.float32, tag="pst")
                        for ci in range(cn):
                            nc.tensor.transpose(out=pst[:, ci, :],
                                                in_=o1[:, c0 + ci, :],
                                                identity=ids[:])
                        o1t = sb.tile([64, 4, P], DT.bfloat16, tag="o1t")
                        nc.vector.tensor_scalar_add(o1t[:, 0:cn, :],
                                                    pst[:, 0:cn, :], -1.0)
                        for ci in range(cn):
                            nc.tensor.matmul(
                                out=t2p[:, c0 + ci, :],
                                lhsT=o1t[:, ci, :], rhs=w2cs[:],
                                start=True, stop=True)
                    t2c = sb.tile([P, CB, 16], DT.bfloat16, tag="t2c")
                    nc.vector.tensor_copy(t2c[:, 0:bn, :], t2p[:, 0:bn, :])
                    nc.vector.tensor_copy(sdc2[:, b0:b0 + bn, :],
                                          t2c[:, 0:bn, 0:11])
                    u2s = sb.tile([P, CB, 11], DT.bfloat16, tag="u2s")
                    y2 = sb.tile([P, CB, 1], DT.float32, tag="y2")
                    nc.vector.tensor_tensor(
                        out=y2[:, 0:bn, :], in0=t2c[:, 0:bn, 0:1],
                        in1=t2c[:, 0:bn, 1:2], op=OP.add)
                    nc.vector.scalar_tensor_tensor(
                        out=y2[:, 0:bn, :], in0=y2[:, 0:bn, :], scalar=NEG,
                        in1=y2[:, 0:bn, :], op0=OP.mult, op1=OP.max)
                    nc.scalar.activation(
                        u2s[:, 0:bn, 10:11], y2[:, 0:bn, :], AF.Exp)
                    nc.vector.tensor_tensor(
                        out=u2s[:, 0:bn, 0:10], in0=t2c[:, 0:bn, 2:12],
                        in1=u2s[:, 0:bn, 10:11].to_broadcast([P, bn, 10]),
                        op=OP.mult)
                    nc.gpsimd.dma_scatter_add(
                        out_ap=u2acc[:, 0:11], in_ap=u2s[:, 0:bn, :],
                        idxs_ap=sfis[:, 8 * b0:8 * (b0 + bn)],
                        num_idxs=bn * P, num_idxs_reg=bn * P, elem_size=11,
                        elem_step=P, queue_num=1, single_packet=False)
                    nc.sync.dma_start(
                        t2sp[b0 * P:(b0 + bn) * P, 0:16]
                        .rearrange("(q p) r -> p q r", q=bn),
                        t2c[:, 0:bn, :])
                pr2 = cp.tile([1, 2], DT.bfloat16)
                nc.vector.memset(pr2[:], -100.0)
                nc.sync.dma_start(t2sp[nslot - 1:nslot, 0:2], pr2[:])

                # ---- phase D: AllGather T2 + s_dst2 quarter tables
                if "D" not in phases:
                    raise _PhaseStop
                if use_cc:
                    nc.gpsimd.collective_compute(
                        "AllGather", OP.bypass, replica_groups=cc_groups,
                        ins=[t2sp[:, :]], outs=[t2fp[:, :]])
                else:
                    for i in range(ncores):
                        nc.sync.dma_start(t2fp[i * nslot:(i + 1) * nslot, :],
                                          t2sp[:, :])
                sd2q = cp.tile([P, NBtot, 2], DT.bfloat16)
                for k in range(nq):
                    for b0 in range(0, nband, 24):
                        bn = min(24, nband - b0)
                        nc.gpsimd.dma_scatter_add(
                            out_ap=sdqt2[:, k * P:k * P + 11],
                            in_ap=sdc2[:, b0:b0 + bn, :],
                            idxs_ap=sdqis[:, 8 * (nband * k + b0):
                                          8 * (nband * k + b0 + bn)],
                            num_idxs=bn * P, num_idxs_reg=bn * P,
                            elem_size=11, elem_step=nq * P, queue_num=1, single_packet=False)
                    nc.sync.dma_start(
                        sd2q[:, qband[k]:qband[k] + NBq[k], :],
                        sdqt2[0:NBq[k] * P, k * P:k * P + 2]
                        .rearrange("(t p) r -> p t r", p=P))

                # ---- phase E: layer-2 edge pipeline
                if "E" not in phases:
                    raise _PhaseStop
                for k in range(min(nq, QMAX)):
                    for (o0, sg, lg, t0) in groups[k][:CMAX]:
                        S = sg * lg
                        it2 = sb.tile([P, 8 * S], DT.int16, tag="it")
                        nc.sync.dma_start(
                            it2[:], idxe[:, 8 * (qcol[k] + o0):
                                         8 * (qcol[k] + o0 + S)])
                        g2 = sb.tile([P, sg, lg, P], DT.bfloat16, tag="g1")
                        nc.gpsimd.dma_gather(
                            out_ap=g2[:].rearrange("p b l r -> p (b l) r"),
                            in_ap=gsrc2[k * qrows:(k + 1) * qrows, :],
                            idxs_ap=it2[:], num_idxs=S * P,
                            num_idxs_reg=S * P, elem_size=P, queue_num=0, single_packet=False)
                        et2 = sb.tile([P, sg, lg], DT.float32, tag="et")
                        nc.vector.tensor_tensor(
                            out=et2[:], in0=g2[:, :, :, 0],
                            in1=sd2q[:, qband[k] + t0:qband[k] + t0 + sg, 1:2]
                                .to_broadcast([P, sg, lg]),
                            op=OP.add)
                        lr2 = sb.tile([P, sg, lg], DT.float32, tag="lr")
                        nc.vector.scalar_tensor_tensor(
                            out=lr2[:], in0=et2[:], scalar=NEG, in1=et2[:],
                            op0=OP.mult, op1=OP.max)
                        p2 = sb.tile([P, sg, lg], DT.bfloat16, tag="p1")
                        nc.scalar.activation(
                            p2[:].rearrange("p b l -> p (b l)"),
                            lr2[:].rearrange("p b l -> p (b l)"), AF.Exp)
                        m2 = sb.tile([P, sg, 10, lg], DT.bfloat16, tag="m1")
                        nc.vector.tensor_tensor(
                            out=m2[:],
                            in0=g2[:, :, :, 2:12]
                                .rearrange("p b l c -> p b c l"),
                            in1=p2[:, :, None, :].to_broadcast([P, sg, 10, lg]),
                            op=OP.mult)
                        u2row = sb.tile([P, sg, 11], DT.bfloat16, tag="urow")
                        with nc.allow_low_precision(reason="bf16 partials"):
                            nc.vector.tensor_reduce(
                                u2row[:, :, 0:10], m2[:], axis=AX.X, op=OP.add)
                            nc.vector.tensor_reduce(
                                u2row[:, :, 10:11], p2[:], axis=AX.X, op=OP.add)
                        rt2 = sb.tile([P, 8 * sg], DT.int16, tag="rt")
                        nc.sync.dma_start(
                            rt2[:], reali[:, 8 * (qband[k] + t0):
                                          8 * (qband[k] + t0 + sg)])
                        nc.gpsimd.dma_scatter_add(
                            out_ap=u2acc[:, 0:11], in_ap=u2row[:],
                            idxs_ap=rt2[:], num_idxs=sg * P,
                            num_idxs_reg=sg * P, elem_size=11,
                            elem_step=P, queue_num=1, single_packet=False)

                # ---- phase F: normalize + per-graph one-hot pooling
                un2 = cp.tile([P, nband, 11], DT.bfloat16)
                nc.sync.dma_start(
                    un2[:], u2acc[:, 0:11].rearrange("(t p) r -> p t r", p=P))
                pps = pq.tile([ng, 12], DT.float32)
                FB = 14
                for b0 in range(0, nband, FB):
                    bn = min(FB, nband - b0)
                    d2 = sb.tile([P, FB, 1], DT.float32, tag="d2")
                    nc.vector.tensor_scalar_add(d2[:, 0:bn, :],
                                                un2[:, b0:b0 + bn, 10:11],
                                                1e-16)
                    r2 = sb.tile([P, FB, 1], DT.float32, tag="r2")
                    nc.vector.reciprocal(r2[:, 0:bn, :], d2[:, 0:bn, :])
                    rhsp = sb.tile([P, FB, 12], DT.bfloat16, tag="rhsp")
                    nc.vector.memset(rhsp[:, 0:bn, 10:11], 1.0)
                    nc.vector.memset(rhsp[:, 0:bn, 11:12], 0.0)
                    nc.vector.tensor_tensor(
                        out=rhsp[:, 0:bn, 0:10], in0=un2[:, b0:b0 + bn, 0:10],
                        in1=r2[:, 0:bn, :].to_broadcast([P, bn, 10]),
                        op=OP.mult)
                    sbh = sb.tile([P, FB, ng], DT.bfloat16, tag="sbh")
                    nc.vector.tensor_tensor(
                        out=sbh[:, 0:bn, :],
                        in0=blis[:, b0:b0 + bn, None].to_broadcast([P, bn, ng]),
                        in1=iogs[:, None, :].to_broadcast([P, bn, ng]),
                        op=OP.is_equal)
                    for b in range(bn):
                        tg = b0 + b
                        nc.tensor.matmul(out=pps[:], lhsT=sbh[:, b, :],
                                         rhs=rhsp[:, b, :],
                                         start=(tg == 0),
                                         stop=(tg == nband - 1),
                                         tile_position=(0, 0))

                po = cp.tile([ng, 12], DT.float32)
                nc.vector.tensor_copy(po[:], pps[:])
                nc.sync.dma_start(pool[:, :], po[:])

    nc.compile()
    return nc


# ---------------------------------------------------------------- launcher
class Launcher:
    def __init__(self, nc, n_cores):
        install_neuronx_cc_hook()
        self.nc = nc
        self.n_cores = n_cores
        pname = nc.partition_id_tensor.name if nc.partition_id_tensor else None
        in_names, out_names, out_avals, zero_outs = [], [], [], []
        for alloc in nc.m.functions[0].allocations:
            if not isinstance(alloc, mybir.MemoryLocationSet):
                continue
            name = alloc.memorylocations[0].name
            if alloc.kind == "ExternalInput":
                if name != pname:
                    in_names.append(name)
            elif alloc.kind == "ExternalOutput":
                out_names.append(name)
                shape = tuple(alloc.tensor_shape)
                dtype = mybir.dt.np(alloc.dtype)
                out_avals.append(jax.core.ShapedArray(shape, dtype))
                zero_outs.append(np.zeros(shape, dtype))
        self.in_names, self.out_names = in_names, out_names
        self.out_avals, self.zero_outs = out_avals, zero_outs
        n_params, n_outs = len(in_names), len(out_avals)
        all_in = in_names + out_names + ([pname] if pname else [])

        def _body(*args):
            operands = list(args)
            if pname is not None:
                operands.append(partition_id_tensor())
            return tuple(_bass_exec_p.bind(
                *operands, out_avals=tuple(out_avals), in_names=tuple(all_in),
                out_names=tuple(out_names), lowering_input_output_aliases=(),
                sim_require_finite=True, sim_require_nnan=True, nc=nc))

        devices = jax.devices()[:n_cores]
        self.mesh = Mesh(np.asarray(devices), ("core",))
        specs_in = (PartitionSpec("core"),) * (n_params + n_outs)
        specs_out = (PartitionSpec("core"),) * n_outs
        self.fn = jax.jit(shard_map(_body, mesh=self.mesh, in_specs=specs_in,
                                    out_specs=specs_out, check_rep=False),
                          keep_unused=True)
        self.sharding = NamedSharding(self.mesh, PartitionSpec("core"))

    def put(self, arr_percore):
        a = np.ascontiguousarray(arr_percore)
        return jax.device_put(a.reshape(a.shape[0] * a.shape[1], *a.shape[2:]),
                              self.sharding)

    def __call__(self, named_args):
        args = [named_args[n] for n in self.in_names]
        for z in self.zero_outs:
            zz = np.zeros((self.n_cores * z.shape[0], *z.shape[1:]), z.dtype)
            args.append(jax.device_put(zz, self.sharding))
        outs = self.fn(*args)
        return dict(zip(self.out_names, outs))


# ---------------------------------------------------------------- host side
_CACHE = {}


def make_inputs(x, edge_index, batch, W1, a1s, a1d, b1, W2, a2s, a2d, cfg, ep):
    n, ncores, npc = cfg["n"], cfg["ncores"], cfg["npc"]
    nslot, nband, ng = cfg["nslot"], cfg["nband"], cfg["ngraph"]
    wf, w2c = build_weights(W1, a1s, a1d, W2, a2s, a2d)

    # xts: per-core xT columns in canonical slot order (dummies -> 0)
    xtp = np.zeros((P, n + 1), np.float32)
    xtp[:, :n] = np.asarray(x, np.float32).T
    xtp = xtp.astype(BF16)
    slot_node = ep["slot_node"]                       # [ncores, nslot]
    sidx = np.where(slot_node >= 0, slot_node, n)
    xts = np.stack([xtp[:, sidx[c]] for c in range(ncores)])

    batch = np.asarray(batch, np.int64)
    bl_flat = np.where(slot_node >= 0,
                       batch[np.maximum(slot_node, 0)], 200)
    bl = np.ascontiguousarray(
        bl_flat.reshape(ncores, nband, P).transpose(0, 2, 1)).astype(BF16)

    rep = lambda a: np.broadcast_to(a, (ncores, *a.shape)).copy()
    iog = np.broadcast_to(np.arange(ng, dtype=np.float32).astype(BF16),
                          (P, ng)).copy()
    b1b = np.broadcast_to(np.asarray(b1, np.float32), (P, 64)).copy()
    ident = np.eye(P, dtype=np.float32)

    return {
        "xts": xts,
        "wf": rep(wf.astype(BF16)),
        "w2c": rep(w2c.astype(BF16)),
        "idxe": ep["idxe"],
        "reali": ep["reali"],
        "sdqsi": ep["sdqsi"],
        "selfi": ep["selfi"],
        "bli": bl,
        "iog": rep(iog),
        "b1i": rep(b1b),
        "idi": rep(ident),
    }


def finish(pool_parts, b2, ng):
    acc = pool_parts.astype(np.float64).sum(axis=0)
    sums = acc[:, :10]
    cnts = np.maximum(acc[:, 10], 1.0)
    pooled = (sums / cnts[:, None] + np.asarray(b2, np.float64)).astype(np.float32)
    m = pooled.max(axis=1, keepdims=True)
    z = pooled - m
    return (z - np.log(np.exp(z).sum(axis=1, keepdims=True))).astype(np.float32)


def kernel(x, edge_index, batch, W1, att_src1, att_dst1, b1,
           W2, att_src2, att_dst2, b2):
    cfg = FULL
    ep = prep_edges(edge_index, cfg)
    key = (tuple(tuple(map(tuple, g)) for g in ep["groups"]),
           bool(np.any(np.asarray(b1))))
    if key not in _CACHE:
        nc = build_module(cfg, ep, use_cc=True,
                          has_b1=bool(np.any(np.asarray(b1))))
        _CACHE[key] = Launcher(nc, cfg["ncores"])
    lau = _CACHE[key]

    named = make_inputs(x, edge_index, batch, W1, att_src1, att_dst1, b1,
                        W2, att_src2, att_dst2, cfg, ep)
    named = {k: lau.put(v) for k, v in named.items()}
    outs = lau(named)
    pool = np.asarray(outs["pool"]).reshape(cfg["ncores"], cfg["ngraph"], 12)
    return finish(pool, b2, cfg["ngraph"])
